# revision 14
# baseline (speedup 1.0000x reference)
"""Differential multi-head attention Trainium2 Bass kernel.

Problem: B=4, N=1024, D=512, H=8 heads, DH=64. LAM=0.5.
  q = (x@Wq+bq)  -> [B,H,N,2*DH], halves q1,q2 (same for k)
  a_i = softmax(q_i@k_i^T / sqrt(DH)); attn = a1 - LAM*a2; out = attn@v

Sharding: 8 cores; core c handles batch b=c//2 and heads h0..h0+3 with
h0=(c%2)*4 (batch + head-group parallel). Weights column-sharded by head.

v3 schedule (all bf16 — fp8 was tried and rejected: operand rounding
error transfers ~1:1 to output relative error, and fp8e4m3's ~4-6%
blows the 2e-2 gate; bf16's ~0.4% gives ~3e-3).

Measured engine model on this platform: a 512-free matmul costs
~0.4-0.6us (PE effectively ~1.2GHz sustained + ~150-200ns/instr);
total PE streaming is the bottleneck, ACT exp (~60us busy at
2048-wide) hides underneath.
  - score matmuls for the two softmax halves are emitted adjacently;
    their lhsT base partitions (0/64) land on different PE row groups
    and run concurrently (~1.4x measured).
  - exp is one 2048-wide ACTIVATE per key tile covering both halves,
    written as bf16 into per-kt e tiles [128,2048].
  - PV: per-kt matmuls; v is augmented with a +1/-2 constant column
    producing softmax denominators in the same accumulation
    (u1/s1 + u2/(-2 s2) = a1@v - 0.5 a2@v).
  - one global period loop (32 periods = 4 heads x 8 key tiles); each
    period emits 4 score MMs + 1 exp, then pumps FIFO job queues with
    per-period budgets: PV 4, next-head projection 2, finish 1.
    PV groups (half,qc) accumulate 8 kt-MMs half-sequentially (the
    second half drains into the next head's periods) so only two
    1-bank PSUM accumulators are ever open.
PSUM (8 banks): scores 1x[128,2048]=4, pv 2x[65,512]=2, proj 1, tr 1.
"""
import hashlib
import os
import sys

sys.path.insert(0, "/opt/trn_rl_repo")

# The libneuronxla NEFF cache keys on the HLO module hash, which does NOT
# change when only the Bass kernel body changes (the custom call carries a
# content-free token) — a shared cache silently reuses stale NEFFs from
# earlier kernel versions. Key the cache dir by this file's content hash.
with open(__file__, "rb") as _f:
    _KSRC = _f.read()
os.environ["NEURON_COMPILE_CACHE_URL"] = (
    f"/tmp/neuron-cache-{hashlib.sha1(_KSRC).hexdigest()[:12]}"
)

from contextlib import ExitStack

import numpy as np

import concourse.bass as bass
import concourse.mybir as mybir
import concourse.tile as tile
from concourse import bacc, bass_utils
from concourse.masks import make_identity

F32 = mybir.dt.float32
BF16 = mybir.dt.bfloat16

B, N, D, H = 4, 1024, 512, 8
DH = 64            # per-head dim for v and per q/k half
HPC = 4            # heads per core
LAM = 0.5
SCALE = 0.125      # 1/sqrt(DH)
NCORES = 8
CQ = HPC * 2 * DH  # 512 projection cols per core for q/k
CV = HPC * DH      # 256 projection cols per core for v
P = 128
NT = N // P        # 8 key tiles
NPAIR = NT // 2    # 4 key-tile pairs (DoubleRow granularity)
DC = D // P        # 4 contraction chunks
QW = 512           # query chunk width (PSUM bank = 512 fp32)
QC = N // QW       # 2 query chunks
AUG = DH + 1       # v columns + constant column
XWDT = BF16
QKDT = BF16


class Job:
    __slots__ = ("min_g", "fn")

    def __init__(self, min_g, fn):
        self.min_g = min_g
        self.fn = fn


def pump(queue, g, budget):
    n = 0
    while queue and n < budget and queue[0].min_g <= g:
        queue.pop(0).fn()
        n += 1


def build_nc(reps=1):
    nc = bacc.Bacc("TRN2", target_bir_lowering=False, debug=False,
                   num_devices=NCORES)
    d = {
        "xt": nc.dram_tensor("xt", [D, N], XWDT, kind="ExternalInput"),
        "wq": nc.dram_tensor("wq", [D, CQ], XWDT, kind="ExternalInput"),
        "wk": nc.dram_tensor("wk", [D, CQ], XWDT, kind="ExternalInput"),
        "wv": nc.dram_tensor("wv", [D, CV], XWDT, kind="ExternalInput"),
        "bq": nc.dram_tensor("bq", [P, HPC], F32, kind="ExternalInput"),
        "bk": nc.dram_tensor("bk", [P, HPC], F32, kind="ExternalInput"),
        "bvb": nc.dram_tensor("bvb", [P, CV], F32, kind="ExternalInput"),
        "o": nc.dram_tensor("o", [N, CV], F32, kind="ExternalOutput"),
    }
    with tile.TileContext(nc) as tc, ExitStack() as ctx:
        consts = ctx.enter_context(tc.tile_pool(name="consts", bufs=1))
        qk = ctx.enter_context(tc.tile_pool(name="qk", bufs=2))
        vaugp = ctx.enter_context(tc.tile_pool(name="vaugp", bufs=1))
        ep = ctx.enter_context(tc.tile_pool(name="ep", bufs=16))
        up = ctx.enter_context(tc.tile_pool(name="up", bufs=3))
        outp = ctx.enter_context(tc.tile_pool(name="outp", bufs=1))
        smallp = ctx.enter_context(tc.tile_pool(name="smallp", bufs=2))
        ps_proj = ctx.enter_context(
            tc.tile_pool(name="ps_proj", bufs=1, space="PSUM"))
        ps_score = ctx.enter_context(
            tc.tile_pool(name="ps_score", bufs=1, space="PSUM"))
        ps_pv = ctx.enter_context(
            tc.tile_pool(name="ps_pv", bufs=1, space="PSUM"))
        ps_tr = ctx.enter_context(
            tc.tile_pool(name="ps_tr", bufs=1, space="PSUM"))

        def body():
            # ---- input DMAs
            xt_sb, wq_sb, wk_sb, wv_sb = [], [], [], []
            for dc in range(DC):
                t = consts.tile([P, N], XWDT, tag=f"xt{dc}", name=f"xt{dc}")
                nc.sync.dma_start(t[:], d["xt"][dc * P:(dc + 1) * P, :])
                xt_sb.append(t)
                t = consts.tile([P, CQ], XWDT, tag=f"wq{dc}", name=f"wq{dc}")
                nc.sync.dma_start(t[:], d["wq"][dc * P:(dc + 1) * P, :])
                wq_sb.append(t)
                t = consts.tile([P, CQ], XWDT, tag=f"wk{dc}", name=f"wk{dc}")
                nc.sync.dma_start(t[:], d["wk"][dc * P:(dc + 1) * P, :])
                wk_sb.append(t)
            for dc in range(DC):
                t = consts.tile([P, CV], XWDT, tag=f"wv{dc}", name=f"wv{dc}")
                nc.sync.dma_start(t[:], d["wv"][dc * P:(dc + 1) * P, :])
                wv_sb.append(t)
            bq_sb = consts.tile([P, HPC], F32, tag="bq", name="bq")
            nc.sync.dma_start(bq_sb[:], d["bq"][:])
            bk_sb = consts.tile([P, HPC], F32, tag="bk", name="bk")
            nc.sync.dma_start(bk_sb[:], d["bk"][:])
            bvb_sb = consts.tile([P, CV], F32, tag="bvb", name="bvb")
            nc.sync.dma_start(bvb_sb[:], d["bvb"][:])
            ident = consts.tile([P, P], F32, tag="ident", name="ident")
            make_identity(nc, ident[:])

            # ---- v projection + augmentation (+1 / -2 constant columns)
            # vaug[nt][half]: [128, HPC*AUG] bf16, head h at [h*AUG : +AUG]
            vaug = [[vaugp.tile([P, HPC * AUG], BF16,
                                tag=f"vaug{nt}_{half}",
                                name=f"vaug{nt}_{half}")
                     for half in range(2)] for nt in range(NT)]
            for nt in range(NT):
                ps = ps_proj.tile([P, CV], F32, tag="proj", name=f"ps_v{nt}")
                for dc in range(DC):
                    nc.tensor.matmul(
                        ps[:],
                        xt_sb[dc][:, nt * P:(nt + 1) * P],
                        wv_sb[dc][:],
                        start=(dc == 0), stop=(dc == DC - 1))
                psv = ps[:].rearrange("p (h a) -> p h a", a=DH)
                bvv = bvb_sb[:].rearrange("p (h a) -> p h a", a=DH)
                for half in range(2):
                    tv = vaug[nt][half][:].rearrange(
                        "p (h a) -> p h a", a=AUG)
                    nc.vector.tensor_add(tv[:, :, 0:DH], psv, bvv)
                    nc.vector.memset(tv[:, :, DH:AUG],
                                     1.0 if half == 0 else -2.0)

            # ---- output staging tiles
            ostage = [outp.tile([P, CV], F32, tag=f"ost{q}", name=f"ost{q}")
                      for q in range(NT)]

            proj_q, pv_q, fin_q = [], [], []

            # ---- projection jobs for one head's q/k (16 PE MMs; the 4th
            # of each accumulation group also does the bias add)
            def push_proj_jobs(h, qt, kt_, min_g):
                for w_sb, b_sb, dest, pfx in ((wq_sb, bq_sb, qt, "q"),
                                              (wk_sb, bk_sb, kt_, "k")):
                    for qc in range(QC):
                        ps = [None]

                        def mk(dc, w_sb=w_sb, b_sb=b_sb, dest=dest, pfx=pfx,
                               qc=qc, ps=ps):
                            def job():
                                if dc == 0:
                                    ps[0] = ps_proj.tile(
                                        [P, QW], F32, tag="proj",
                                        name=f"ps_{pfx}{h}_{qc}")
                                nc.tensor.matmul(
                                    ps[0][:],
                                    w_sb[dc][:, h * P:(h + 1) * P],
                                    xt_sb[dc][:, qc * QW:(qc + 1) * QW],
                                    start=(dc == 0), stop=(dc == DC - 1))
                                if dc == DC - 1:
                                    nc.vector.tensor_scalar_add(
                                        dest[:, qc * QW:(qc + 1) * QW],
                                        ps[0][:], b_sb[:, h:h + 1])
                            return job
                        for dc in range(DC):
                            proj_q.append(Job(min_g, mk(dc)))

            # ---- finish jobs for head h (one per query tile)
            def push_fin_jobs(h, u_pair, min_g):
                u1, u2 = u_pair

                def mk(qt_i):
                    def job():
                        tr = ps_tr.tile([P, 2 * AUG], F32, tag="tr",
                                        name=f"ps_tr{h}_{qt_i}")
                        nc.tensor.transpose(
                            tr[:, 0:AUG],
                            u1[0:AUG, qt_i * P:(qt_i + 1) * P],
                            ident[0:AUG, 0:AUG])
                        nc.tensor.transpose(
                            tr[:, AUG:2 * AUG],
                            u2[0:AUG, qt_i * P:(qt_i + 1) * P],
                            ident[0:AUG, 0:AUG])
                        rr = smallp.tile([P, 2], F32, tag="rr",
                                         name=f"rr_{h}_{qt_i}")
                        trv = tr[:].rearrange("p (c a) -> p c a", a=AUG)
                        nc.vector.reciprocal(rr[:], trv[:, :, DH])
                        o1 = smallp.tile([P, DH], F32, tag="o1",
                                         name=f"o1_{h}{qt_i}")
                        o2 = smallp.tile([P, DH], F32, tag="o2",
                                         name=f"o2_{h}{qt_i}")
                        nc.vector.tensor_scalar_mul(
                            o1[:], tr[:, 0:DH], rr[:, 0:1])
                        nc.vector.tensor_scalar_mul(
                            o2[:], tr[:, AUG:AUG + DH], rr[:, 1:2])
                        nc.vector.tensor_add(
                            ostage[qt_i][:, h * DH:(h + 1) * DH], o1[:], o2[:])
                        if h == HPC - 1:
                            nc.sync.dma_start(
                                d["o"][qt_i * P:(qt_i + 1) * P, :],
                                ostage[qt_i][:])
                    return job
                for qt_i in range(NT):
                    fin_q.append(Job(min_g, mk(qt_i)))

            # ---- PV jobs for head h: 32 MMs, groups (half,qc) accumulate
            # kt 0..7; half-sequential so only two 1-bank accumulators are
            # open (half 1 drains into the next head's periods).
            # e tile [128, 2048]: [h0q0|h1q0|h0q1|h1q1] x 512.
            def push_pv_jobs(h, etiles, u_tiles, base_g):
                pvst = {}

                def mk(half, kt, qc):
                    def job():
                        if kt == 0:
                            pvst[(half, qc)] = ps_pv.tile(
                                [AUG, QW], F32, tag=f"pv{qc}",
                                name=f"ps_pv{h}_{half}_{qc}")
                        pvt = pvst[(half, qc)]
                        nc.tensor.matmul(
                            pvt[:],
                            vaug[kt][half][:, h * AUG:(h + 1) * AUG],
                            etiles[kt][:, (half + 2 * qc) * QW:
                                       (half + 2 * qc) * QW + QW],
                            start=(kt == 0), stop=(kt == NT - 1))
                        if kt == NT - 1:
                            nc.vector.tensor_copy(
                                u_tiles[half][:, qc * QW:(qc + 1) * QW],
                                pvt[:])
                            if half == 1 and qc == QC - 1:
                                push_fin_jobs(h, (u_tiles[0], u_tiles[1]),
                                              cur_g[0] + 1)
                    return job
                for half in range(2):
                    for kt in range(NT):
                        for qc in range(QC):
                            # e(kt) ready after the ACT emitted at
                            # period base_g + kt
                            pv_q.append(Job(base_g + kt + 1,
                                            mk(half, kt, qc)))

            cur_g = [0]

            # ---- head 0 projections up front
            qt, kt_, = (qk.tile([P, N], QKDT, tag="qt", name="qt0"),
                        qk.tile([P, N], QKDT, tag="kt", name="kt0"))
            push_proj_jobs(0, qt, kt_, min_g=-1)
            while proj_q and proj_q[0].min_g <= -1:
                proj_q.pop(0).fn()

            heads = {0: (qt, kt_)}
            for g in range(HPC * NT):
                h, kt = g // NT, g % NT
                cur_g[0] = g
                if kt == 0:
                    # allocate next head's qt/kt and queue its projections
                    if h + 1 < HPC:
                        nqt = qk.tile([P, N], QKDT, tag="qt", name=f"qt{h+1}")
                        nkt = qk.tile([P, N], QKDT, tag="kt", name=f"kt{h+1}")
                        heads[h + 1] = (nqt, nkt)
                        push_proj_jobs(h + 1, nqt, nkt, min_g=g)
                    # e tiles + u tiles + pv jobs for this head
                    etiles = [ep.tile([P, 2048], BF16, tag="e",
                                      name=f"e{h}_{k}")
                              for k in range(NT)]
                    u_tiles = [up.tile([AUG, N], F32, tag=f"u{hf}",
                                       name=f"u{h}_{hf}")
                               for hf in range(2)]
                    push_pv_jobs(h, etiles, u_tiles, base_g=g)
                qt, kt_ = heads[h]
                ps = ps_score.tile([P, 2048], F32, tag="score",
                                   name=f"s{h}_{kt}")
                for qc in range(QC):
                    for half in range(2):
                        nc.tensor.matmul(
                            ps[:, (half + 2 * qc) * QW:
                               (half + 2 * qc) * QW + QW],
                            kt_[half * DH:(half + 1) * DH,
                                kt * P:(kt + 1) * P],
                            qt[half * DH:(half + 1) * DH,
                               qc * QW:(qc + 1) * QW],
                            start=True, stop=True)
                nc.scalar.activation(
                    etiles[kt][:], ps[:],
                    mybir.ActivationFunctionType.Exp, scale=SCALE)
                pump(pv_q, g, 4)
                pump(proj_q, g, 2)
                if kt >= 1:
                    pump(fin_q, g, 1)
            cur_g[0] = 10 ** 6
            while pv_q:
                pv_q.pop(0).fn()
            while fin_q:
                fin_q.pop(0).fn()

        if reps == 1:
            body()
        else:
            with tc.For_i(0, reps, 1,
                          hint_engines=(mybir.EngineType.PE,
                                        mybir.EngineType.DVE)):
                body()

    nc.compile()
    return nc


_NC_CACHE = {}


def get_nc(reps=1):
    if reps not in _NC_CACHE:
        _NC_CACHE[reps] = build_nc(reps)
    return _NC_CACHE[reps]


def shard_inputs(inputs):
    import ml_dtypes
    xw_np = (np.dtype(ml_dtypes.bfloat16)
             if XWDT == mybir.dt.bfloat16 else np.float32)
    x = np.asarray(inputs["x"], dtype=np.float32)
    Wq = np.asarray(inputs["Wq"], dtype=np.float32)
    bq = np.asarray(inputs["bq"], dtype=np.float32)
    Wk = np.asarray(inputs["Wk"], dtype=np.float32)
    bk = np.asarray(inputs["bk"], dtype=np.float32)
    Wv = np.asarray(inputs["Wv"], dtype=np.float32)
    bv = np.asarray(inputs["bv"], dtype=np.float32)
    in_maps = []
    for c in range(NCORES):
        b = c // 2
        h0 = (c % 2) * HPC
        cq0 = h0 * 2 * DH          # 0 or 512 in the q/k projection cols
        cv0 = h0 * DH              # 0 or 256 in the v cols
        in_maps.append({
            "xt": np.ascontiguousarray(x[b].T).astype(xw_np),
            "wq": np.ascontiguousarray(Wq[:, cq0:cq0 + CQ]).astype(xw_np),
            "wk": np.ascontiguousarray(Wk[:, cq0:cq0 + CQ]).astype(xw_np),
            "wv": np.ascontiguousarray(Wv[:, cv0:cv0 + CV]).astype(xw_np),
            "bq": np.ascontiguousarray(bq[cq0:cq0 + CQ].reshape(HPC, P).T),
            "bk": np.ascontiguousarray(bk[cq0:cq0 + CQ].reshape(HPC, P).T),
            "bvb": np.ascontiguousarray(
                np.broadcast_to(bv[cv0:cv0 + CV], (P, CV))),
        })
    return in_maps


def assemble_output(results):
    out = np.empty((B, N, D), dtype=np.float32)
    for c in range(NCORES):
        b = c // 2
        g = c % 2
        out[b, :, g * CV:(g + 1) * CV] = results[c]["o"]
    return out


def kernel(**inputs):
    nc = get_nc(1)
    in_maps = shard_inputs(inputs)
    res = bass_utils.run_bass_kernel_spmd(
        nc, in_maps, core_ids=list(range(NCORES)))
    return assemble_output(res.results)


# revision 16
# speedup vs baseline: 1.3207x; 1.3207x over previous
"""Differential multi-head attention Trainium2 Bass kernel.

Problem: B=4, N=1024, D=512, H=8 heads, DH=64. LAM=0.5.
  q = (x@Wq+bq)  -> [B,H,N,2*DH], halves q1,q2 (same for k)
  a_i = softmax(q_i@k_i^T / sqrt(DH)); attn = a1 - LAM*a2; out = attn@v

Sharding: 8 cores; core c handles batch b=c//2 and heads h0..h0+3 with
h0=(c%2)*4 (batch + head-group parallel). Weights column-sharded by head.

v3 schedule (all bf16 — fp8 was tried and rejected: operand rounding
error transfers ~1:1 to output relative error, and fp8e4m3's ~4-6%
blows the 2e-2 gate; bf16's ~0.4% gives ~3e-3).

Measured engine model on this platform: a 512-free matmul costs
~0.4-0.6us (PE effectively ~1.2GHz sustained + ~150-200ns/instr);
total PE streaming is the bottleneck, ACT exp (~60us busy at
2048-wide) hides underneath.
  - score matmuls for the two softmax halves are emitted adjacently;
    their lhsT base partitions (0/64) land on different PE row groups
    and run concurrently (~1.4x measured).
  - exp is one 2048-wide ACTIVATE per key tile covering both halves,
    written as bf16 into per-kt e tiles [128,2048].
  - PV: per-kt matmuls; v is augmented with a +1/-2 constant column
    producing softmax denominators in the same accumulation
    (u1/s1 + u2/(-2 s2) = a1@v - 0.5 a2@v).
  - one global period loop (32 periods = 4 heads x 8 key tiles); each
    period emits 4 score MMs + 1 exp, then pumps FIFO job queues with
    per-period budgets: PV 4, next-head projection 2, finish 1.
    PV groups (half,qc) accumulate 8 kt-MMs half-sequentially (the
    second half drains into the next head's periods) so only two
    1-bank PSUM accumulators are ever open.
PSUM (8 banks): scores 1x[128,2048]=4, pv 2x[65,512]=2, proj 1, tr 1.
"""
import hashlib
import os
import sys

sys.path.insert(0, "/opt/trn_rl_repo")

# The libneuronxla NEFF cache keys on the HLO module hash, which does NOT
# change when only the Bass kernel body changes (the custom call carries a
# content-free token) — a shared cache silently reuses stale NEFFs from
# earlier kernel versions. Key the cache dir by this file's content hash.
with open(__file__, "rb") as _f:
    _KSRC = _f.read()
os.environ["NEURON_COMPILE_CACHE_URL"] = (
    f"/tmp/neuron-cache-{hashlib.sha1(_KSRC).hexdigest()[:12]}"
)

from contextlib import ExitStack

import numpy as np

import concourse.bass as bass
import concourse.mybir as mybir
import concourse.tile as tile
from concourse import bacc, bass_utils
from concourse.masks import make_identity

F32 = mybir.dt.float32
BF16 = mybir.dt.bfloat16

B, N, D, H = 4, 1024, 512, 8
DH = 64            # per-head dim for v and per q/k half
HPC = 4            # heads per core
LAM = 0.5
SCALE = 0.125      # 1/sqrt(DH)
NCORES = 8
CQ = HPC * 2 * DH  # 512 projection cols per core for q/k
CV = HPC * DH      # 256 projection cols per core for v
P = 128
NT = N // P        # 8 key tiles
NPAIR = NT // 2    # 4 key-tile pairs (DoubleRow granularity)
DC = D // P        # 4 contraction chunks
QW = 512           # query chunk width (PSUM bank = 512 fp32)
QC = N // QW       # 2 query chunks
AUG = DH + 1       # v columns + constant column
XWDT = BF16
QKDT = BF16


class Job:
    __slots__ = ("min_g", "fn")

    def __init__(self, min_g, fn):
        self.min_g = min_g
        self.fn = fn


def pump(queue, g, budget):
    n = 0
    while queue and n < budget and queue[0].min_g <= g:
        queue.pop(0).fn()
        n += 1


def build_nc(reps=1):
    nc = bacc.Bacc("TRN2", target_bir_lowering=False, debug=False,
                   num_devices=NCORES)
    d = {
        "xt": nc.dram_tensor("xt", [D, N], XWDT, kind="ExternalInput"),
        "wq": nc.dram_tensor("wq", [D, CQ], XWDT, kind="ExternalInput"),
        "wk": nc.dram_tensor("wk", [D, CQ], XWDT, kind="ExternalInput"),
        "wv": nc.dram_tensor("wv", [D, CV], XWDT, kind="ExternalInput"),
        "bq": nc.dram_tensor("bq", [P, HPC], F32, kind="ExternalInput"),
        "bk": nc.dram_tensor("bk", [P, HPC], F32, kind="ExternalInput"),
        "bvb": nc.dram_tensor("bvb", [P, CV], F32, kind="ExternalInput"),
        "o": nc.dram_tensor("o", [N, CV], F32, kind="ExternalOutput"),
    }
    with tile.TileContext(nc) as tc, ExitStack() as ctx:
        consts = ctx.enter_context(tc.tile_pool(name="consts", bufs=1))
        qk = ctx.enter_context(tc.tile_pool(name="qk", bufs=2))
        vaugp = ctx.enter_context(tc.tile_pool(name="vaugp", bufs=1))
        ep = ctx.enter_context(tc.tile_pool(name="ep", bufs=16))
        up = ctx.enter_context(tc.tile_pool(name="up", bufs=3))
        outp = ctx.enter_context(tc.tile_pool(name="outp", bufs=1))
        smallp = ctx.enter_context(tc.tile_pool(name="smallp", bufs=2))
        ps_proj = ctx.enter_context(
            tc.tile_pool(name="ps_proj", bufs=1, space="PSUM"))
        ps_score = ctx.enter_context(
            tc.tile_pool(name="ps_score", bufs=2, space="PSUM"))
        ps_pv = ctx.enter_context(
            tc.tile_pool(name="ps_pv", bufs=1, space="PSUM"))
        ps_tr = ctx.enter_context(
            tc.tile_pool(name="ps_tr", bufs=1, space="PSUM"))

        def body():
            # ---- input DMAs
            xt_sb, wq_sb, wk_sb, wv_sb = [], [], [], []
            for dc in range(DC):
                t = consts.tile([P, N], XWDT, tag=f"xt{dc}", name=f"xt{dc}")
                nc.sync.dma_start(t[:], d["xt"][dc * P:(dc + 1) * P, :])
                xt_sb.append(t)
                t = consts.tile([P, CQ], XWDT, tag=f"wq{dc}", name=f"wq{dc}")
                nc.sync.dma_start(t[:], d["wq"][dc * P:(dc + 1) * P, :])
                wq_sb.append(t)
                t = consts.tile([P, CQ], XWDT, tag=f"wk{dc}", name=f"wk{dc}")
                nc.sync.dma_start(t[:], d["wk"][dc * P:(dc + 1) * P, :])
                wk_sb.append(t)
            for dc in range(DC):
                t = consts.tile([P, CV], XWDT, tag=f"wv{dc}", name=f"wv{dc}")
                nc.sync.dma_start(t[:], d["wv"][dc * P:(dc + 1) * P, :])
                wv_sb.append(t)
            bq_sb = consts.tile([P, HPC], F32, tag="bq", name="bq")
            nc.sync.dma_start(bq_sb[:], d["bq"][:])
            bk_sb = consts.tile([P, HPC], F32, tag="bk", name="bk")
            nc.sync.dma_start(bk_sb[:], d["bk"][:])
            bvb_sb = consts.tile([P, CV], F32, tag="bvb", name="bvb")
            nc.sync.dma_start(bvb_sb[:], d["bvb"][:])
            ident = consts.tile([P, P], F32, tag="ident", name="ident")
            make_identity(nc, ident[:])

            # ---- v projection + augmentation (+1 / -2 constant columns)
            # vaug[nt][half]: [128, HPC*AUG] bf16, head h at [h*AUG : +AUG]
            vaug = [[vaugp.tile([P, HPC * AUG], BF16,
                                tag=f"vaug{nt}_{half}",
                                name=f"vaug{nt}_{half}")
                     for half in range(2)] for nt in range(NT)]
            for nt in range(NT):
                ps = ps_proj.tile([P, CV], F32, tag="proj", name=f"ps_v{nt}")
                for dc in range(DC):
                    nc.tensor.matmul(
                        ps[:],
                        xt_sb[dc][:, nt * P:(nt + 1) * P],
                        wv_sb[dc][:],
                        start=(dc == 0), stop=(dc == DC - 1))
                psv = ps[:].rearrange("p (h a) -> p h a", a=DH)
                bvv = bvb_sb[:].rearrange("p (h a) -> p h a", a=DH)
                for half in range(2):
                    tv = vaug[nt][half][:].rearrange(
                        "p (h a) -> p h a", a=AUG)
                    nc.vector.tensor_add(tv[:, :, 0:DH], psv, bvv)
                    nc.vector.memset(tv[:, :, DH:AUG],
                                     1.0 if half == 0 else -2.0)

            # ---- output staging tiles
            ostage = [outp.tile([P, CV], F32, tag=f"ost{q}", name=f"ost{q}")
                      for q in range(NT)]

            proj_q, pv_q, fin_q = [], [], []

            # ---- projection jobs for one head's q/k (16 PE MMs; the 4th
            # of each accumulation group also does the bias add)
            def push_proj_jobs(h, qt, kt_, min_g):
                for w_sb, b_sb, dest, pfx in ((wq_sb, bq_sb, qt, "q"),
                                              (wk_sb, bk_sb, kt_, "k")):
                    for qc in range(QC):
                        ps = [None]

                        def mk(dc, w_sb=w_sb, b_sb=b_sb, dest=dest, pfx=pfx,
                               qc=qc, ps=ps):
                            def job():
                                if dc == 0:
                                    ps[0] = ps_proj.tile(
                                        [P, QW], F32, tag="proj",
                                        name=f"ps_{pfx}{h}_{qc}")
                                nc.tensor.matmul(
                                    ps[0][:],
                                    w_sb[dc][:, h * P:(h + 1) * P],
                                    xt_sb[dc][:, qc * QW:(qc + 1) * QW],
                                    start=(dc == 0), stop=(dc == DC - 1))
                                if dc == DC - 1:
                                    nc.vector.tensor_scalar_add(
                                        dest[:, qc * QW:(qc + 1) * QW],
                                        ps[0][:], b_sb[:, h:h + 1])
                            return job
                        for dc in range(DC):
                            proj_q.append(Job(min_g, mk(dc)))

            # ---- finish jobs for head h (one per query tile)
            def push_fin_jobs(h, u_pair, min_g):
                u1, u2 = u_pair

                def mk(qt_i):
                    def job():
                        tr = ps_tr.tile([P, 2 * AUG], F32, tag="tr",
                                        name=f"ps_tr{h}_{qt_i}")
                        nc.tensor.transpose(
                            tr[:, 0:AUG],
                            u1[0:AUG, qt_i * P:(qt_i + 1) * P],
                            ident[0:AUG, 0:AUG])
                        nc.tensor.transpose(
                            tr[:, AUG:2 * AUG],
                            u2[0:AUG, qt_i * P:(qt_i + 1) * P],
                            ident[0:AUG, 0:AUG])
                        rr = smallp.tile([P, 2], F32, tag="rr",
                                         name=f"rr_{h}_{qt_i}")
                        trv = tr[:].rearrange("p (c a) -> p c a", a=AUG)
                        nc.vector.reciprocal(rr[:], trv[:, :, DH])
                        o1 = smallp.tile([P, DH], F32, tag="o1",
                                         name=f"o1_{h}{qt_i}")
                        o2 = smallp.tile([P, DH], F32, tag="o2",
                                         name=f"o2_{h}{qt_i}")
                        nc.vector.tensor_scalar_mul(
                            o1[:], tr[:, 0:DH], rr[:, 0:1])
                        nc.vector.tensor_scalar_mul(
                            o2[:], tr[:, AUG:AUG + DH], rr[:, 1:2])
                        nc.vector.tensor_add(
                            ostage[qt_i][:, h * DH:(h + 1) * DH], o1[:], o2[:])
                        if h == HPC - 1:
                            nc.sync.dma_start(
                                d["o"][qt_i * P:(qt_i + 1) * P, :],
                                ostage[qt_i][:])
                    return job
                for qt_i in range(NT):
                    fin_q.append(Job(min_g, mk(qt_i)))

            # ---- PV jobs for head h: 32 MMs, groups (half,qc) accumulate
            # kt 0..7; half-sequential so only two 1-bank accumulators are
            # open (half 1 drains into the next head's periods).
            # e tile [128, 2048]: [h0q0|h1q0|h0q1|h1q1] x 512.
            def push_pv_jobs(h, etiles, u_tiles, base_g):
                pvst = {}

                def mk(half, kt, qc):
                    def job():
                        if kt == 0:
                            pvst[(half, qc)] = ps_pv.tile(
                                [AUG, QW], F32, tag=f"pv{qc}",
                                name=f"ps_pv{h}_{half}_{qc}")
                        pvt = pvst[(half, qc)]
                        nc.tensor.matmul(
                            pvt[:],
                            vaug[kt][half][:, h * AUG:(h + 1) * AUG],
                            etiles[kt][:, (half + 2 * qc) * QW:
                                       (half + 2 * qc) * QW + QW],
                            start=(kt == 0), stop=(kt == NT - 1))
                        if kt == NT - 1:
                            nc.vector.tensor_copy(
                                u_tiles[half][:, qc * QW:(qc + 1) * QW],
                                pvt[:])
                            if half == 1 and qc == QC - 1:
                                push_fin_jobs(h, (u_tiles[0], u_tiles[1]),
                                              cur_g[0] + 1)
                    return job
                for half in range(2):
                    for kt in range(NT):
                        for qc in range(QC):
                            # e(kt) ready after the ACT emitted at
                            # period base_g + kt
                            pv_q.append(Job(base_g + kt + 1,
                                            mk(half, kt, qc)))

            cur_g = [0]

            # ---- head 0 projections up front
            qt, kt_, = (qk.tile([P, N], QKDT, tag="qt", name="qt0"),
                        qk.tile([P, N], QKDT, tag="kt", name="kt0"))
            push_proj_jobs(0, qt, kt_, min_g=-1)
            while proj_q and proj_q[0].min_g <= -1:
                proj_q.pop(0).fn()

            heads = {0: (qt, kt_)}
            for g in range(HPC * NT):
                h, kt = g // NT, g % NT
                cur_g[0] = g
                if kt == 0:
                    # allocate next head's qt/kt and queue its projections
                    if h + 1 < HPC:
                        nqt = qk.tile([P, N], QKDT, tag="qt", name=f"qt{h+1}")
                        nkt = qk.tile([P, N], QKDT, tag="kt", name=f"kt{h+1}")
                        heads[h + 1] = (nqt, nkt)
                        push_proj_jobs(h + 1, nqt, nkt, min_g=g)
                    # e tiles + u tiles + pv jobs for this head
                    etiles = [ep.tile([P, 2048], BF16, tag="e",
                                      name=f"e{h}_{k}")
                              for k in range(NT)]
                    u_tiles = [up.tile([AUG, N], F32, tag=f"u{hf}",
                                       name=f"u{h}_{hf}")
                               for hf in range(2)]
                    push_pv_jobs(h, etiles, u_tiles, base_g=g)
                qt, kt_ = heads[h]
                for qc in range(QC):
                    ps = ps_score.tile([P, 1024], F32, tag="score",
                                       name=f"s{h}_{kt}_{qc}")
                    for half in range(2):
                        nc.tensor.matmul(
                            ps[:, half * QW:half * QW + QW],
                            kt_[half * DH:(half + 1) * DH,
                                kt * P:(kt + 1) * P],
                            qt[half * DH:(half + 1) * DH,
                               qc * QW:(qc + 1) * QW],
                            start=True, stop=True)
                    nc.scalar.activation(
                        etiles[kt][:, qc * 1024:(qc + 1) * 1024], ps[:],
                        mybir.ActivationFunctionType.Exp, scale=SCALE)
                pump(pv_q, g, 4)
                pump(proj_q, g, 2)
                if kt >= 1:
                    pump(fin_q, g, 1)
            cur_g[0] = 10 ** 6
            while pv_q:
                pv_q.pop(0).fn()
            while fin_q:
                fin_q.pop(0).fn()

        if reps == 1:
            body()
        else:
            with tc.For_i(0, reps, 1,
                          hint_engines=(mybir.EngineType.PE,
                                        mybir.EngineType.DVE)):
                body()

    nc.compile()
    return nc


_NC_CACHE = {}


def get_nc(reps=1):
    if reps not in _NC_CACHE:
        _NC_CACHE[reps] = build_nc(reps)
    return _NC_CACHE[reps]


def shard_inputs(inputs):
    import ml_dtypes
    xw_np = (np.dtype(ml_dtypes.bfloat16)
             if XWDT == mybir.dt.bfloat16 else np.float32)
    x = np.asarray(inputs["x"], dtype=np.float32)
    Wq = np.asarray(inputs["Wq"], dtype=np.float32)
    bq = np.asarray(inputs["bq"], dtype=np.float32)
    Wk = np.asarray(inputs["Wk"], dtype=np.float32)
    bk = np.asarray(inputs["bk"], dtype=np.float32)
    Wv = np.asarray(inputs["Wv"], dtype=np.float32)
    bv = np.asarray(inputs["bv"], dtype=np.float32)
    in_maps = []
    for c in range(NCORES):
        b = c // 2
        h0 = (c % 2) * HPC
        cq0 = h0 * 2 * DH          # 0 or 512 in the q/k projection cols
        cv0 = h0 * DH              # 0 or 256 in the v cols
        in_maps.append({
            "xt": np.ascontiguousarray(x[b].T).astype(xw_np),
            "wq": np.ascontiguousarray(Wq[:, cq0:cq0 + CQ]).astype(xw_np),
            "wk": np.ascontiguousarray(Wk[:, cq0:cq0 + CQ]).astype(xw_np),
            "wv": np.ascontiguousarray(Wv[:, cv0:cv0 + CV]).astype(xw_np),
            "bq": np.ascontiguousarray(bq[cq0:cq0 + CQ].reshape(HPC, P).T),
            "bk": np.ascontiguousarray(bk[cq0:cq0 + CQ].reshape(HPC, P).T),
            "bvb": np.ascontiguousarray(
                np.broadcast_to(bv[cv0:cv0 + CV], (P, CV))),
        })
    return in_maps


def assemble_output(results):
    out = np.empty((B, N, D), dtype=np.float32)
    for c in range(NCORES):
        b = c // 2
        g = c % 2
        out[b, :, g * CV:(g + 1) * CV] = results[c]["o"]
    return out


def kernel(**inputs):
    nc = get_nc(1)
    in_maps = shard_inputs(inputs)
    res = bass_utils.run_bass_kernel_spmd(
        nc, in_maps, core_ids=list(range(NCORES)))
    return assemble_output(res.results)


# revision 20
# speedup vs baseline: 1.4123x; 1.0694x over previous
"""Differential multi-head attention Trainium2 Bass kernel.

Problem: B=4, N=1024, D=512, H=8 heads, DH=64. LAM=0.5.
  q = (x@Wq+bq)  -> [B,H,N,2*DH], halves q1,q2 (same for k)
  a_i = softmax(q_i@k_i^T / sqrt(DH)); attn = a1 - LAM*a2; out = attn@v

Sharding: 8 cores; core c handles batch b=c//2 and heads h0..h0+3 with
h0=(c%2)*4 (batch + head-group parallel). Weights column-sharded by head.

v3 schedule (all bf16 — fp8 was tried and rejected: operand rounding
error transfers ~1:1 to output relative error, and fp8e4m3's ~4-6%
blows the 2e-2 gate; bf16's ~0.4% gives ~3e-3).

Measured engine model on this platform: a 512-free matmul costs
~0.4-0.6us (PE effectively ~1.2GHz sustained + ~150-200ns/instr);
total PE streaming is the bottleneck, ACT exp (~60us busy at
2048-wide) hides underneath.
  - score matmuls for the two softmax halves are emitted adjacently;
    their lhsT base partitions (0/64) land on different PE row groups
    and run concurrently (~1.4x measured).
  - exp is one 2048-wide ACTIVATE per key tile covering both halves,
    written as bf16 into per-kt e tiles [128,2048].
  - PV: per-kt matmuls; v is augmented with a +1/-2 constant column
    producing softmax denominators in the same accumulation
    (u1/s1 + u2/(-2 s2) = a1@v - 0.5 a2@v).
  - one global period loop (32 periods = 4 heads x 8 key tiles); each
    period emits 4 score MMs + 1 exp, then pumps FIFO job queues with
    per-period budgets: PV 4, next-head projection 2, finish 1.
    PV groups (half,qc) accumulate 8 kt-MMs half-sequentially (the
    second half drains into the next head's periods) so only two
    1-bank PSUM accumulators are ever open.
PSUM (8 banks): scores 1x[128,2048]=4, pv 2x[65,512]=2, proj 1, tr 1.
"""
import hashlib
import os
import sys

sys.path.insert(0, "/opt/trn_rl_repo")

# The libneuronxla NEFF cache keys on the HLO module hash, which does NOT
# change when only the Bass kernel body changes (the custom call carries a
# content-free token) — a shared cache silently reuses stale NEFFs from
# earlier kernel versions. Key the cache dir by this file's content hash.
with open(__file__, "rb") as _f:
    _KSRC = _f.read()
os.environ["NEURON_COMPILE_CACHE_URL"] = (
    f"/tmp/neuron-cache-{hashlib.sha1(_KSRC).hexdigest()[:12]}"
)

from contextlib import ExitStack

import numpy as np

import concourse.bass as bass
import concourse.mybir as mybir
import concourse.tile as tile
from concourse import bacc, bass_utils
from concourse.masks import make_identity

F32 = mybir.dt.float32
BF16 = mybir.dt.bfloat16

B, N, D, H = 4, 1024, 512, 8
DH = 64            # per-head dim for v and per q/k half
HPC = 4            # heads per core
LAM = 0.5
SCALE = 0.125      # 1/sqrt(DH)
NCORES = 8
CQ = HPC * 2 * DH  # 512 projection cols per core for q/k
CV = HPC * DH      # 256 projection cols per core for v
P = 128
NT = N // P        # 8 key tiles
NPAIR = NT // 2    # 4 key-tile pairs (DoubleRow granularity)
DC = D // P        # 4 contraction chunks
QW = 512           # query chunk width (PSUM bank = 512 fp32)
QC = N // QW       # 2 query chunks
AUG = DH + 1       # v columns + constant column
XWDT = BF16
QKDT = BF16


class Job:
    __slots__ = ("min_g", "fn")

    def __init__(self, min_g, fn):
        self.min_g = min_g
        self.fn = fn


def pump(queue, g, budget):
    n = 0
    while queue and n < budget and queue[0].min_g <= g:
        queue.pop(0).fn()
        n += 1


def build_nc(reps=1, skip=()):
    skip = set(skip)
    nc = bacc.Bacc("TRN2", target_bir_lowering=False, debug=False,
                   num_devices=NCORES)
    d = {
        "xt": nc.dram_tensor("xt", [D, N], XWDT, kind="ExternalInput"),
        "wq": nc.dram_tensor("wq", [D, CQ], XWDT, kind="ExternalInput"),
        "wk": nc.dram_tensor("wk", [D, CQ], XWDT, kind="ExternalInput"),
        "wv": nc.dram_tensor("wv", [D, CV], XWDT, kind="ExternalInput"),
        "bq": nc.dram_tensor("bq", [P, HPC], F32, kind="ExternalInput"),
        "bk": nc.dram_tensor("bk", [P, HPC], F32, kind="ExternalInput"),
        "bvb": nc.dram_tensor("bvb", [P, CV], F32, kind="ExternalInput"),
        "o": nc.dram_tensor("o", [N, CV], F32, kind="ExternalOutput"),
    }
    with tile.TileContext(nc) as tc, ExitStack() as ctx:
        consts = ctx.enter_context(tc.tile_pool(name="consts", bufs=1))
        qk = ctx.enter_context(tc.tile_pool(name="qk", bufs=2))
        vaugp = ctx.enter_context(tc.tile_pool(name="vaugp", bufs=1))
        ep = ctx.enter_context(tc.tile_pool(name="ep", bufs=16))
        up = ctx.enter_context(tc.tile_pool(name="up", bufs=3))
        outp = ctx.enter_context(tc.tile_pool(name="outp", bufs=1))
        smallp = ctx.enter_context(tc.tile_pool(name="smallp", bufs=2))
        ps_proj = ctx.enter_context(
            tc.tile_pool(name="ps_proj", bufs=1, space="PSUM"))
        ps_score = ctx.enter_context(
            tc.tile_pool(name="ps_score", bufs=2, space="PSUM"))
        ps_pv = ctx.enter_context(
            tc.tile_pool(name="ps_pv", bufs=1, space="PSUM"))
        ps_tr = ctx.enter_context(
            tc.tile_pool(name="ps_tr", bufs=1, space="PSUM"))

        def body():
            # ---- input DMAs
            xt_sb, wq_sb, wk_sb, wv_sb = [], [], [], []
            for dc in range(DC):
                t = consts.tile([P, N], XWDT, tag=f"xt{dc}", name=f"xt{dc}")
                nc.sync.dma_start(t[:], d["xt"][dc * P:(dc + 1) * P, :])
                xt_sb.append(t)
                t = consts.tile([P, CQ], XWDT, tag=f"wq{dc}", name=f"wq{dc}")
                nc.sync.dma_start(t[:], d["wq"][dc * P:(dc + 1) * P, :])
                wq_sb.append(t)
                t = consts.tile([P, CQ], XWDT, tag=f"wk{dc}", name=f"wk{dc}")
                nc.sync.dma_start(t[:], d["wk"][dc * P:(dc + 1) * P, :])
                wk_sb.append(t)
            for dc in range(DC):
                t = consts.tile([P, CV], XWDT, tag=f"wv{dc}", name=f"wv{dc}")
                nc.sync.dma_start(t[:], d["wv"][dc * P:(dc + 1) * P, :])
                wv_sb.append(t)
            bq_sb = consts.tile([P, HPC], F32, tag="bq", name="bq")
            nc.sync.dma_start(bq_sb[:], d["bq"][:])
            bk_sb = consts.tile([P, HPC], F32, tag="bk", name="bk")
            nc.sync.dma_start(bk_sb[:], d["bk"][:])
            bvb_sb = consts.tile([P, CV], F32, tag="bvb", name="bvb")
            nc.sync.dma_start(bvb_sb[:], d["bvb"][:])
            ident = consts.tile([P, P], F32, tag="ident", name="ident")
            make_identity(nc, ident[:])

            # ---- v projection + augmentation (+1 / -2 constant columns)
            # vaug[nt][half]: [128, HPC*AUG] bf16, head h at [h*AUG : +AUG]
            vaug = [[vaugp.tile([P, HPC * AUG], BF16,
                                tag=f"vaug{nt}_{half}",
                                name=f"vaug{nt}_{half}")
                     for half in range(2)] for nt in range(NT)]
            vproj_q = []

            def mk_vproj(nt):
                def job():
                    ps = ps_proj.tile([P, CV], F32, tag="proj",
                                      name=f"ps_v{nt}")
                    for dc in range(DC):
                        nc.tensor.matmul(
                            ps[:],
                            xt_sb[dc][:, nt * P:(nt + 1) * P],
                            wv_sb[dc][:],
                            start=(dc == 0), stop=(dc == DC - 1))
                    psv = ps[:].rearrange("p (h a) -> p h a", a=DH)
                    bvv = bvb_sb[:].rearrange("p (h a) -> p h a", a=DH)
                    for half in range(2):
                        tv = vaug[nt][half][:].rearrange(
                            "p (h a) -> p h a", a=AUG)
                        nc.vector.tensor_add(tv[:, :, 0:DH], psv, bvv)
                        nc.vector.memset(tv[:, :, DH:AUG],
                                         1.0 if half == 0 else -2.0)
                return job
            for nt in range(NT):
                vproj_q.append(Job(-1, mk_vproj(nt)))

            # ---- output staging tiles
            ostage = [outp.tile([P, CV], F32, tag=f"ost{q}", name=f"ost{q}")
                      for q in range(NT)]

            proj_q, pv_q, fin_q = [], [], []

            # ---- projection jobs for one head's q/k (16 PE MMs; the 4th
            # of each accumulation group also does the bias add)
            def push_proj_jobs(h, qt, kt_, min_g):
                for w_sb, b_sb, dest, pfx in ((wq_sb, bq_sb, qt, "q"),
                                              (wk_sb, bk_sb, kt_, "k")):
                    for qc in range(QC):
                        ps = [None]

                        def mk(dc, w_sb=w_sb, b_sb=b_sb, dest=dest, pfx=pfx,
                               qc=qc, ps=ps):
                            def job():
                                if dc == 0:
                                    ps[0] = ps_proj.tile(
                                        [P, QW], F32, tag="proj",
                                        name=f"ps_{pfx}{h}_{qc}")
                                nc.tensor.matmul(
                                    ps[0][:],
                                    w_sb[dc][:, h * P:(h + 1) * P],
                                    xt_sb[dc][:, qc * QW:(qc + 1) * QW],
                                    start=(dc == 0), stop=(dc == DC - 1))
                                if dc == DC - 1:
                                    nc.vector.tensor_scalar_add(
                                        dest[:, qc * QW:(qc + 1) * QW],
                                        ps[0][:], b_sb[:, h:h + 1])
                            return job
                        for dc in range(DC):
                            if "proj" not in skip:
                                proj_q.append(Job(min_g, mk(dc)))

            # ---- finish jobs for head h (one per query tile)
            def push_fin_jobs(h, u_pair, min_g):
                u1, u2 = u_pair

                def mk(qt_i):
                    def job():
                        tr = ps_tr.tile([P, 2 * AUG], F32, tag="tr",
                                        name=f"ps_tr{h}_{qt_i}")
                        nc.tensor.transpose(
                            tr[:, 0:AUG],
                            u1[0:AUG, qt_i * P:(qt_i + 1) * P],
                            ident[0:AUG, 0:AUG])
                        nc.tensor.transpose(
                            tr[:, AUG:2 * AUG],
                            u2[0:AUG, qt_i * P:(qt_i + 1) * P],
                            ident[0:AUG, 0:AUG])
                        rr = smallp.tile([P, 2], F32, tag="rr",
                                         name=f"rr_{h}_{qt_i}")
                        trv = tr[:].rearrange("p (c a) -> p c a", a=AUG)
                        nc.vector.reciprocal(rr[:], trv[:, :, DH])
                        o1 = smallp.tile([P, DH], F32, tag="o1",
                                         name=f"o1_{h}{qt_i}")
                        o2 = smallp.tile([P, DH], F32, tag="o2",
                                         name=f"o2_{h}{qt_i}")
                        nc.vector.tensor_scalar_mul(
                            o1[:], tr[:, 0:DH], rr[:, 0:1])
                        nc.vector.tensor_scalar_mul(
                            o2[:], tr[:, AUG:AUG + DH], rr[:, 1:2])
                        nc.vector.tensor_add(
                            ostage[qt_i][:, h * DH:(h + 1) * DH], o1[:], o2[:])
                        if h == HPC - 1:
                            nc.sync.dma_start(
                                d["o"][qt_i * P:(qt_i + 1) * P, :],
                                ostage[qt_i][:])
                    return job
                for qt_i in range(NT):
                    if "fin" not in skip:
                        fin_q.append(Job(min_g, mk(qt_i)))

            # ---- PV jobs for head h: 32 MMs, groups (half,qc) accumulate
            # kt 0..7; half-sequential so only two 1-bank accumulators are
            # open (half 1 drains into the next head's periods).
            # e tile [128, 2048]: [h0q0|h1q0|h0q1|h1q1] x 512.
            def push_pv_jobs(h, etiles, u_tiles, base_g):
                pvst = {}

                def mk(half, kt, qc):
                    def job():
                        if kt == 0:
                            pvst[(half, qc)] = ps_pv.tile(
                                [AUG, QW], F32, tag=f"pv{qc}",
                                name=f"ps_pv{h}_{half}_{qc}")
                        pvt = pvst[(half, qc)]
                        nc.tensor.matmul(
                            pvt[:],
                            vaug[kt][half][:, h * AUG:(h + 1) * AUG],
                            etiles[kt][:, (half + 2 * qc) * QW:
                                       (half + 2 * qc) * QW + QW],
                            start=(kt == 0), stop=(kt == NT - 1))
                        if kt == NT - 1:
                            nc.vector.tensor_copy(
                                u_tiles[half][:, qc * QW:(qc + 1) * QW],
                                pvt[:])
                            if half == 1 and qc == QC - 1:
                                push_fin_jobs(h, (u_tiles[0], u_tiles[1]),
                                              cur_g[0] + 1)
                    return job
                for half in range(2):
                    for kt in range(NT):
                        for qc in range(QC):
                            # e(kt) ready after the ACT emitted at
                            # period base_g + kt
                            if "pv" not in skip:
                                pv_q.append(Job(base_g + kt + 1,
                                                mk(half, kt, qc)))

            cur_g = [0]

            # ---- head 0 projections up front
            qt, kt_, = (qk.tile([P, N], QKDT, tag="qt", name="qt0"),
                        qk.tile([P, N], QKDT, tag="kt", name="kt0"))
            if "proj" in skip:
                nc.vector.memset(qt[:], 0.0)
                nc.vector.memset(kt_[:], 0.0)
            push_proj_jobs(0, qt, kt_, min_g=-1)
            while proj_q and proj_q[0].min_g <= -1:
                proj_q.pop(0).fn()

            heads = {0: (qt, kt_)}
            for g in range(HPC * NT):
                h, kt = g // NT, g % NT
                cur_g[0] = g
                if kt == 0:
                    # allocate next head's qt/kt and queue its projections
                    if h + 1 < HPC:
                        nqt = qk.tile([P, N], QKDT, tag="qt", name=f"qt{h+1}")
                        nkt = qk.tile([P, N], QKDT, tag="kt", name=f"kt{h+1}")
                        heads[h + 1] = (nqt, nkt)
                        if "proj" in skip:
                            nc.vector.memset(nqt[:], 0.0)
                            nc.vector.memset(nkt[:], 0.0)
                        push_proj_jobs(h + 1, nqt, nkt, min_g=g)
                    # e tiles + u tiles + pv jobs for this head
                    etiles = [ep.tile([P, 2048], BF16, tag="e",
                                      name=f"e{h}_{k}")
                              for k in range(NT)]
                    if "score" in skip and "pv" not in skip:
                        for t in etiles:
                            nc.vector.memset(t[:], 1.0)
                    u_tiles = [up.tile([AUG, N], F32, tag=f"u{hf}",
                                       name=f"u{h}_{hf}")
                               for hf in range(2)]
                    push_pv_jobs(h, etiles, u_tiles, base_g=g)
                qt, kt_ = heads[h]
                for qc in range(QC):
                    if "score" in skip:
                        break
                    ps = ps_score.tile([P, 1024], F32, tag="score",
                                       name=f"s{h}_{kt}_{qc}")
                    for half in range(2):
                        for _rep in range(2 if "dup" in skip else 1):
                            nc.tensor.matmul(
                                ps[:, half * QW:half * QW + QW],
                                kt_[half * DH:(half + 1) * DH,
                                    kt * P:(kt + 1) * P],
                                qt[half * DH:(half + 1) * DH,
                                   qc * QW:(qc + 1) * QW],
                                start=True, stop=True)
                    if "act" not in skip:
                        nc.scalar.activation(
                            etiles[kt][:, qc * 1024:(qc + 1) * 1024], ps[:],
                            mybir.ActivationFunctionType.Exp, scale=SCALE)
                pump(vproj_q, g, 1)
                pump(pv_q, g, 4)
                pump(proj_q, g, 2)
                if kt >= 1:
                    pump(fin_q, g, 1)
            cur_g[0] = 10 ** 6
            while pv_q:
                pv_q.pop(0).fn()
            while fin_q:
                fin_q.pop(0).fn()

        if reps == 1:
            body()
        else:
            with tc.For_i(0, reps, 1,
                          hint_engines=(mybir.EngineType.PE,
                                        mybir.EngineType.DVE)):
                body()

    nc.compile()
    return nc


_NC_CACHE = {}


def get_nc(reps=1):
    if reps not in _NC_CACHE:
        _NC_CACHE[reps] = build_nc(reps)
    return _NC_CACHE[reps]


def shard_inputs(inputs):
    import ml_dtypes
    xw_np = (np.dtype(ml_dtypes.bfloat16)
             if XWDT == mybir.dt.bfloat16 else np.float32)
    x = np.asarray(inputs["x"], dtype=np.float32)
    Wq = np.asarray(inputs["Wq"], dtype=np.float32)
    bq = np.asarray(inputs["bq"], dtype=np.float32)
    Wk = np.asarray(inputs["Wk"], dtype=np.float32)
    bk = np.asarray(inputs["bk"], dtype=np.float32)
    Wv = np.asarray(inputs["Wv"], dtype=np.float32)
    bv = np.asarray(inputs["bv"], dtype=np.float32)
    in_maps = []
    for c in range(NCORES):
        b = c // 2
        h0 = (c % 2) * HPC
        cq0 = h0 * 2 * DH          # 0 or 512 in the q/k projection cols
        cv0 = h0 * DH              # 0 or 256 in the v cols
        in_maps.append({
            "xt": np.ascontiguousarray(x[b].T).astype(xw_np),
            "wq": np.ascontiguousarray(Wq[:, cq0:cq0 + CQ]).astype(xw_np),
            "wk": np.ascontiguousarray(Wk[:, cq0:cq0 + CQ]).astype(xw_np),
            "wv": np.ascontiguousarray(Wv[:, cv0:cv0 + CV]).astype(xw_np),
            "bq": np.ascontiguousarray(bq[cq0:cq0 + CQ].reshape(HPC, P).T),
            "bk": np.ascontiguousarray(bk[cq0:cq0 + CQ].reshape(HPC, P).T),
            "bvb": np.ascontiguousarray(
                np.broadcast_to(bv[cv0:cv0 + CV], (P, CV))),
        })
    return in_maps


def assemble_output(results):
    out = np.empty((B, N, D), dtype=np.float32)
    for c in range(NCORES):
        b = c // 2
        g = c % 2
        out[b, :, g * CV:(g + 1) * CV] = results[c]["o"]
    return out


def kernel(**inputs):
    nc = get_nc(1)
    in_maps = shard_inputs(inputs)
    res = bass_utils.run_bass_kernel_spmd(
        nc, in_maps, core_ids=list(range(NCORES)))
    return assemble_output(res.results)


# revision 21
# speedup vs baseline: 1.4686x; 1.0399x over previous
"""Differential multi-head attention Trainium2 Bass kernel.

Problem: B=4, N=1024, D=512, H=8 heads, DH=64. LAM=0.5.
  q = (x@Wq+bq)  -> [B,H,N,2*DH], halves q1,q2 (same for k)
  a_i = softmax(q_i@k_i^T / sqrt(DH)); attn = a1 - LAM*a2; out = attn@v

Sharding: 8 cores; core c handles batch b=c//2 and heads h0..h0+3 with
h0=(c%2)*4 (batch + head-group parallel). Weights column-sharded by head.

v3 schedule (all bf16 — fp8 was tried and rejected: operand rounding
error transfers ~1:1 to output relative error, and fp8e4m3's ~4-6%
blows the 2e-2 gate; bf16's ~0.4% gives ~3e-3).

Measured engine model on this platform: a 512-free matmul costs
~0.4-0.6us (PE effectively ~1.2GHz sustained + ~150-200ns/instr);
total PE streaming is the bottleneck, ACT exp (~60us busy at
2048-wide) hides underneath.
  - score matmuls for the two softmax halves are emitted adjacently;
    their lhsT base partitions (0/64) land on different PE row groups
    and run concurrently (~1.4x measured).
  - exp is one 2048-wide ACTIVATE per key tile covering both halves,
    written as bf16 into per-kt e tiles [128,2048].
  - PV: per-kt matmuls; v is augmented with a +1/-2 constant column
    producing softmax denominators in the same accumulation
    (u1/s1 + u2/(-2 s2) = a1@v - 0.5 a2@v).
  - one global period loop (32 periods = 4 heads x 8 key tiles); each
    period emits 4 score MMs + 1 exp, then pumps FIFO job queues with
    per-period budgets: PV 4, next-head projection 2, finish 1.
    PV groups (half,qc) accumulate 8 kt-MMs half-sequentially (the
    second half drains into the next head's periods) so only two
    1-bank PSUM accumulators are ever open.
PSUM (8 banks): scores 1x[128,2048]=4, pv 2x[65,512]=2, proj 1, tr 1.
"""
import hashlib
import os
import sys

sys.path.insert(0, "/opt/trn_rl_repo")

# The libneuronxla NEFF cache keys on the HLO module hash, which does NOT
# change when only the Bass kernel body changes (the custom call carries a
# content-free token) — a shared cache silently reuses stale NEFFs from
# earlier kernel versions. Key the cache dir by this file's content hash.
with open(__file__, "rb") as _f:
    _KSRC = _f.read()
os.environ["NEURON_COMPILE_CACHE_URL"] = (
    f"/tmp/neuron-cache-{hashlib.sha1(_KSRC).hexdigest()[:12]}"
)

from contextlib import ExitStack

import numpy as np

import concourse.bass as bass
import concourse.mybir as mybir
import concourse.tile as tile
from concourse import bacc, bass_utils
from concourse.masks import make_identity

F32 = mybir.dt.float32
BF16 = mybir.dt.bfloat16

B, N, D, H = 4, 1024, 512, 8
DH = 64            # per-head dim for v and per q/k half
HPC = 4            # heads per core
LAM = 0.5
SCALE = 0.125      # 1/sqrt(DH)
NCORES = 8
CQ = HPC * 2 * DH  # 512 projection cols per core for q/k
CV = HPC * DH      # 256 projection cols per core for v
P = 128
NT = N // P        # 8 key tiles
NPAIR = NT // 2    # 4 key-tile pairs (DoubleRow granularity)
DC = D // P        # 4 contraction chunks
QW = 512           # query chunk width (PSUM bank = 512 fp32)
QC = N // QW       # 2 query chunks
AUG = DH + 1       # v columns + constant column
XWDT = BF16
QKDT = BF16


class Job:
    __slots__ = ("min_g", "fn")

    def __init__(self, min_g, fn):
        self.min_g = min_g
        self.fn = fn


def pump(queue, g, budget):
    n = 0
    while queue and n < budget and queue[0].min_g <= g:
        queue.pop(0).fn()
        n += 1


def build_nc(reps=1, skip=()):
    skip = set(skip)
    nc = bacc.Bacc("TRN2", target_bir_lowering=False, debug=False,
                   num_devices=NCORES)
    d = {
        "xt": nc.dram_tensor("xt", [D, N], XWDT, kind="ExternalInput"),
        "wq": nc.dram_tensor("wq", [D, CQ], XWDT, kind="ExternalInput"),
        "wk": nc.dram_tensor("wk", [D, CQ], XWDT, kind="ExternalInput"),
        "wv": nc.dram_tensor("wv", [D, CV], XWDT, kind="ExternalInput"),
        "bq": nc.dram_tensor("bq", [P, HPC], F32, kind="ExternalInput"),
        "bk": nc.dram_tensor("bk", [P, HPC], F32, kind="ExternalInput"),
        "bvb": nc.dram_tensor("bvb", [P, CV], F32, kind="ExternalInput"),
        "o": nc.dram_tensor("o", [N, CV], F32, kind="ExternalOutput"),
    }
    with tile.TileContext(nc) as tc, ExitStack() as ctx:
        consts = ctx.enter_context(tc.tile_pool(name="consts", bufs=1))
        qk = ctx.enter_context(tc.tile_pool(name="qk", bufs=2))
        vaugp = ctx.enter_context(tc.tile_pool(name="vaugp", bufs=1))
        ep = ctx.enter_context(tc.tile_pool(name="ep", bufs=16))
        up = ctx.enter_context(tc.tile_pool(name="up", bufs=3))
        outp = ctx.enter_context(tc.tile_pool(name="outp", bufs=1))
        smallp = ctx.enter_context(tc.tile_pool(name="smallp", bufs=2))
        ps_proj = ctx.enter_context(
            tc.tile_pool(name="ps_proj", bufs=1, space="PSUM"))
        ps_score = ctx.enter_context(
            tc.tile_pool(name="ps_score", bufs=2, space="PSUM"))
        ps_pv = ctx.enter_context(
            tc.tile_pool(name="ps_pv", bufs=1, space="PSUM"))
        ps_tr = ctx.enter_context(
            tc.tile_pool(name="ps_tr", bufs=1, space="PSUM"))

        def body():
            # ---- input DMAs
            xt_sb, wq_sb, wk_sb, wv_sb = [], [], [], []
            for dc in range(DC):
                t = consts.tile([P, N], XWDT, tag=f"xt{dc}", name=f"xt{dc}")
                nc.sync.dma_start(t[:], d["xt"][dc * P:(dc + 1) * P, :])
                xt_sb.append(t)
                t = consts.tile([P, CQ], XWDT, tag=f"wq{dc}", name=f"wq{dc}")
                nc.sync.dma_start(t[:], d["wq"][dc * P:(dc + 1) * P, :])
                wq_sb.append(t)
                t = consts.tile([P, CQ], XWDT, tag=f"wk{dc}", name=f"wk{dc}")
                nc.sync.dma_start(t[:], d["wk"][dc * P:(dc + 1) * P, :])
                wk_sb.append(t)
            for dc in range(DC):
                t = consts.tile([P, CV], XWDT, tag=f"wv{dc}", name=f"wv{dc}")
                nc.sync.dma_start(t[:], d["wv"][dc * P:(dc + 1) * P, :])
                wv_sb.append(t)
            bq_sb = consts.tile([P, HPC], F32, tag="bq", name="bq")
            nc.sync.dma_start(bq_sb[:], d["bq"][:])
            bk_sb = consts.tile([P, HPC], F32, tag="bk", name="bk")
            nc.sync.dma_start(bk_sb[:], d["bk"][:])
            bvb_sb = consts.tile([P, CV], F32, tag="bvb", name="bvb")
            nc.sync.dma_start(bvb_sb[:], d["bvb"][:])
            ident = consts.tile([P, P], F32, tag="ident", name="ident")
            make_identity(nc, ident[:])

            # ---- v projection + augmentation (+1 / -2 constant columns)
            # vaug[nt][half]: [128, HPC*AUG] bf16, head h at [h*AUG : +AUG]
            vaug = [[vaugp.tile([P, HPC * AUG], BF16,
                                tag=f"vaug{nt}_{half}",
                                name=f"vaug{nt}_{half}")
                     for half in range(2)] for nt in range(NT)]
            vproj_q = []

            def mk_vproj(nt):
                def job():
                    ps = ps_proj.tile([P, CV], F32, tag="proj",
                                      name=f"ps_v{nt}")
                    for dc in range(DC):
                        nc.tensor.matmul(
                            ps[:],
                            xt_sb[dc][:, nt * P:(nt + 1) * P],
                            wv_sb[dc][:],
                            start=(dc == 0), stop=(dc == DC - 1))
                    psv = ps[:].rearrange("p (h a) -> p h a", a=DH)
                    bvv = bvb_sb[:].rearrange("p (h a) -> p h a", a=DH)
                    for half in range(2):
                        tv = vaug[nt][half][:].rearrange(
                            "p (h a) -> p h a", a=AUG)
                        nc.vector.tensor_add(tv[:, :, 0:DH], psv, bvv)
                        nc.vector.memset(tv[:, :, DH:AUG],
                                         1.0 if half == 0 else -2.0)
                return job
            for nt in range(NT):
                vproj_q.append(Job(-1, mk_vproj(nt)))

            # ---- output staging tiles
            ostage = [outp.tile([P, CV], F32, tag=f"ost{q}", name=f"ost{q}")
                      for q in range(NT)]

            proj_q, pv_q, fin_q = [], [], []

            # ---- projection jobs for one head's q/k (16 PE MMs; the 4th
            # of each accumulation group also does the bias add)
            def push_proj_jobs(h, qt, kt_, min_g):
                for qc in range(QC):
                    for w_sb, b_sb, dest, pfx in ((wq_sb, bq_sb, qt, "q"),
                                                  (wk_sb, bk_sb, kt_, "k")):
                        ps = [None]

                        def mk(dc, w_sb=w_sb, b_sb=b_sb, dest=dest, pfx=pfx,
                               qc=qc, ps=ps):
                            def job():
                                if dc == 0:
                                    ps[0] = ps_proj.tile(
                                        [P, QW], F32, tag="proj",
                                        name=f"ps_{pfx}{h}_{qc}")
                                nc.tensor.matmul(
                                    ps[0][:],
                                    w_sb[dc][:, h * P:(h + 1) * P],
                                    xt_sb[dc][:, qc * QW:(qc + 1) * QW],
                                    start=(dc == 0), stop=(dc == DC - 1))
                                if dc == DC - 1:
                                    nc.vector.tensor_scalar_add(
                                        dest[:, qc * QW:(qc + 1) * QW],
                                        ps[0][:], b_sb[:, h:h + 1])
                            return job
                        for dc in range(DC):
                            if "proj" not in skip:
                                proj_q.append(Job(min_g, mk(dc)))

            # ---- finish jobs for head h (one per query tile)
            def push_fin_jobs(h, u_pair, min_g):
                u1, u2 = u_pair

                def mk(qt_i):
                    def job():
                        tr = ps_tr.tile([P, 2 * AUG], F32, tag="tr",
                                        name=f"ps_tr{h}_{qt_i}")
                        nc.tensor.transpose(
                            tr[:, 0:AUG],
                            u1[0:AUG, qt_i * P:(qt_i + 1) * P],
                            ident[0:AUG, 0:AUG])
                        nc.tensor.transpose(
                            tr[:, AUG:2 * AUG],
                            u2[0:AUG, qt_i * P:(qt_i + 1) * P],
                            ident[0:AUG, 0:AUG])
                        rr = smallp.tile([P, 2], F32, tag="rr",
                                         name=f"rr_{h}_{qt_i}")
                        trv = tr[:].rearrange("p (c a) -> p c a", a=AUG)
                        nc.vector.reciprocal(rr[:], trv[:, :, DH])
                        o1 = smallp.tile([P, DH], F32, tag="o1",
                                         name=f"o1_{h}{qt_i}")
                        o2 = smallp.tile([P, DH], F32, tag="o2",
                                         name=f"o2_{h}{qt_i}")
                        nc.vector.tensor_scalar_mul(
                            o1[:], tr[:, 0:DH], rr[:, 0:1])
                        nc.vector.tensor_scalar_mul(
                            o2[:], tr[:, AUG:AUG + DH], rr[:, 1:2])
                        nc.vector.tensor_add(
                            ostage[qt_i][:, h * DH:(h + 1) * DH], o1[:], o2[:])
                        nc.sync.dma_start(
                            d["o"][qt_i * P:(qt_i + 1) * P,
                                   h * DH:(h + 1) * DH],
                            ostage[qt_i][:, h * DH:(h + 1) * DH])
                    return job
                for qt_i in range(NT):
                    if "fin" not in skip:
                        fin_q.append(Job(min_g, mk(qt_i)))

            # ---- PV jobs for head h: 32 MMs, groups (half,qc) accumulate
            # kt 0..7; half-sequential so only two 1-bank accumulators are
            # open (half 1 drains into the next head's periods).
            # e tile [128, 2048]: [h0q0|h1q0|h0q1|h1q1] x 512.
            def push_pv_jobs(h, etiles, u_tiles, base_g):
                pvst = {}

                def mk(half, kt, qc):
                    def job():
                        if kt == 0:
                            pvst[(half, qc)] = ps_pv.tile(
                                [AUG, QW], F32, tag=f"pv{qc}",
                                name=f"ps_pv{h}_{half}_{qc}")
                        pvt = pvst[(half, qc)]
                        nc.tensor.matmul(
                            pvt[:],
                            vaug[kt][half][:, h * AUG:(h + 1) * AUG],
                            etiles[kt][:, (half + 2 * qc) * QW:
                                       (half + 2 * qc) * QW + QW],
                            start=(kt == 0), stop=(kt == NT - 1))
                        if kt == NT - 1:
                            nc.vector.tensor_copy(
                                u_tiles[half][:, qc * QW:(qc + 1) * QW],
                                pvt[:])
                            if half == 1 and qc == QC - 1:
                                push_fin_jobs(h, (u_tiles[0], u_tiles[1]),
                                              cur_g[0] + 1)
                    return job
                for half in range(2):
                    for kt in range(NT):
                        for qc in range(QC):
                            # e(kt) ready after the ACT emitted at
                            # period base_g + kt
                            if "pv" not in skip:
                                pv_q.append(Job(base_g + kt + 1,
                                                mk(half, kt, qc)))

            cur_g = [0]

            # ---- head 0 projections up front
            qt, kt_, = (qk.tile([P, N], QKDT, tag="qt", name="qt0"),
                        qk.tile([P, N], QKDT, tag="kt", name="kt0"))
            if "proj" in skip:
                nc.vector.memset(qt[:], 0.0)
                nc.vector.memset(kt_[:], 0.0)
            push_proj_jobs(0, qt, kt_, min_g=-1)
            while proj_q and proj_q[0].min_g <= -1:
                proj_q.pop(0).fn()

            heads = {0: (qt, kt_)}
            for g in range(HPC * NT):
                h, kt = g // NT, g % NT
                cur_g[0] = g
                if kt == 0:
                    # allocate next head's qt/kt and queue its projections
                    if h + 1 < HPC:
                        nqt = qk.tile([P, N], QKDT, tag="qt", name=f"qt{h+1}")
                        nkt = qk.tile([P, N], QKDT, tag="kt", name=f"kt{h+1}")
                        heads[h + 1] = (nqt, nkt)
                        if "proj" in skip:
                            nc.vector.memset(nqt[:], 0.0)
                            nc.vector.memset(nkt[:], 0.0)
                        push_proj_jobs(h + 1, nqt, nkt, min_g=g)
                    # e tiles + u tiles + pv jobs for this head
                    etiles = [ep.tile([P, 2048], BF16, tag="e",
                                      name=f"e{h}_{k}")
                              for k in range(NT)]
                    if "score" in skip and "pv" not in skip:
                        for t in etiles:
                            nc.vector.memset(t[:], 1.0)
                    u_tiles = [up.tile([AUG, N], F32, tag=f"u{hf}",
                                       name=f"u{h}_{hf}")
                               for hf in range(2)]
                    push_pv_jobs(h, etiles, u_tiles, base_g=g)
                qt, kt_ = heads[h]
                for qc in range(QC):
                    if "score" in skip:
                        break
                    ps = ps_score.tile([P, 1024], F32, tag="score",
                                       name=f"s{h}_{kt}_{qc}")
                    for half in range(2):
                        for _rep in range(2 if "dup" in skip else 1):
                            nc.tensor.matmul(
                                ps[:, half * QW:half * QW + QW],
                                kt_[half * DH:(half + 1) * DH,
                                    kt * P:(kt + 1) * P],
                                qt[half * DH:(half + 1) * DH,
                                   qc * QW:(qc + 1) * QW],
                                start=True, stop=True)
                    if "act" not in skip:
                        nc.scalar.activation(
                            etiles[kt][:, qc * 1024:(qc + 1) * 1024], ps[:],
                            mybir.ActivationFunctionType.Exp, scale=SCALE)
                pump(vproj_q, g, 1)
                pump(pv_q, g, 6)
                pump(proj_q, g, 2)
                if kt >= 1:
                    pump(fin_q, g, 1)
            cur_g[0] = 10 ** 6
            while pv_q:
                pv_q.pop(0).fn()
            while fin_q:
                fin_q.pop(0).fn()

        if reps == 1:
            body()
        else:
            with tc.For_i(0, reps, 1,
                          hint_engines=(mybir.EngineType.PE,
                                        mybir.EngineType.DVE)):
                body()

    nc.compile()
    return nc


_NC_CACHE = {}


def get_nc(reps=1):
    if reps not in _NC_CACHE:
        _NC_CACHE[reps] = build_nc(reps)
    return _NC_CACHE[reps]


def shard_inputs(inputs):
    import ml_dtypes
    xw_np = (np.dtype(ml_dtypes.bfloat16)
             if XWDT == mybir.dt.bfloat16 else np.float32)
    x = np.asarray(inputs["x"], dtype=np.float32)
    Wq = np.asarray(inputs["Wq"], dtype=np.float32)
    bq = np.asarray(inputs["bq"], dtype=np.float32)
    Wk = np.asarray(inputs["Wk"], dtype=np.float32)
    bk = np.asarray(inputs["bk"], dtype=np.float32)
    Wv = np.asarray(inputs["Wv"], dtype=np.float32)
    bv = np.asarray(inputs["bv"], dtype=np.float32)
    in_maps = []
    for c in range(NCORES):
        b = c // 2
        h0 = (c % 2) * HPC
        cq0 = h0 * 2 * DH          # 0 or 512 in the q/k projection cols
        cv0 = h0 * DH              # 0 or 256 in the v cols
        in_maps.append({
            "xt": np.ascontiguousarray(x[b].T).astype(xw_np),
            "wq": np.ascontiguousarray(Wq[:, cq0:cq0 + CQ]).astype(xw_np),
            "wk": np.ascontiguousarray(Wk[:, cq0:cq0 + CQ]).astype(xw_np),
            "wv": np.ascontiguousarray(Wv[:, cv0:cv0 + CV]).astype(xw_np),
            "bq": np.ascontiguousarray(bq[cq0:cq0 + CQ].reshape(HPC, P).T),
            "bk": np.ascontiguousarray(bk[cq0:cq0 + CQ].reshape(HPC, P).T),
            "bvb": np.ascontiguousarray(
                np.broadcast_to(bv[cv0:cv0 + CV], (P, CV))),
        })
    return in_maps


def assemble_output(results):
    out = np.empty((B, N, D), dtype=np.float32)
    for c in range(NCORES):
        b = c // 2
        g = c % 2
        out[b, :, g * CV:(g + 1) * CV] = results[c]["o"]
    return out


def kernel(**inputs):
    nc = get_nc(1)
    in_maps = shard_inputs(inputs)
    res = bass_utils.run_bass_kernel_spmd(
        nc, in_maps, core_ids=list(range(NCORES)))
    return assemble_output(res.results)


# revision 22
# speedup vs baseline: 1.4732x; 1.0031x over previous
"""Differential multi-head attention Trainium2 Bass kernel.

Problem: B=4, N=1024, D=512, H=8 heads, DH=64. LAM=0.5.
  q = (x@Wq+bq)  -> [B,H,N,2*DH], halves q1,q2 (same for k)
  a_i = softmax(q_i@k_i^T / sqrt(DH)); attn = a1 - LAM*a2; out = attn@v

Sharding: 8 cores; core c handles batch b=c//2 and heads h0..h0+3 with
h0=(c%2)*4 (batch + head-group parallel). Weights column-sharded by head.

v3 schedule (all bf16 — fp8 was tried and rejected: operand rounding
error transfers ~1:1 to output relative error, and fp8e4m3's ~4-6%
blows the 2e-2 gate; bf16's ~0.4% gives ~3e-3).

Measured engine model on this platform: a 512-free matmul costs
~0.4-0.6us (PE effectively ~1.2GHz sustained + ~150-200ns/instr);
total PE streaming is the bottleneck, ACT exp (~60us busy at
2048-wide) hides underneath.
  - score matmuls for the two softmax halves are emitted adjacently;
    their lhsT base partitions (0/64) land on different PE row groups
    and run concurrently (~1.4x measured).
  - exp is one 2048-wide ACTIVATE per key tile covering both halves,
    written as bf16 into per-kt e tiles [128,2048].
  - PV: per-kt matmuls; v is augmented with a +1/-2 constant column
    producing softmax denominators in the same accumulation
    (u1/s1 + u2/(-2 s2) = a1@v - 0.5 a2@v).
  - one global period loop (32 periods = 4 heads x 8 key tiles); each
    period emits 4 score MMs + 1 exp, then pumps FIFO job queues with
    per-period budgets: PV 4, next-head projection 2, finish 1.
    PV groups (half,qc) accumulate 8 kt-MMs half-sequentially (the
    second half drains into the next head's periods) so only two
    1-bank PSUM accumulators are ever open.
PSUM (8 banks): scores 1x[128,2048]=4, pv 2x[65,512]=2, proj 1, tr 1.
"""
import hashlib
import os
import sys

sys.path.insert(0, "/opt/trn_rl_repo")

# The libneuronxla NEFF cache keys on the HLO module hash, which does NOT
# change when only the Bass kernel body changes (the custom call carries a
# content-free token) — a shared cache silently reuses stale NEFFs from
# earlier kernel versions. Key the cache dir by this file's content hash.
with open(__file__, "rb") as _f:
    _KSRC = _f.read()
os.environ["NEURON_COMPILE_CACHE_URL"] = (
    f"/tmp/neuron-cache-{hashlib.sha1(_KSRC).hexdigest()[:12]}"
)

from contextlib import ExitStack

import numpy as np

import concourse.bass as bass
import concourse.mybir as mybir
import concourse.tile as tile
from concourse import bacc, bass_utils
from concourse.masks import make_identity

F32 = mybir.dt.float32
BF16 = mybir.dt.bfloat16

B, N, D, H = 4, 1024, 512, 8
DH = 64            # per-head dim for v and per q/k half
HPC = 4            # heads per core
LAM = 0.5
SCALE = 0.125      # 1/sqrt(DH)
NCORES = 8
CQ = HPC * 2 * DH  # 512 projection cols per core for q/k
CV = HPC * DH      # 256 projection cols per core for v
P = 128
NT = N // P        # 8 key tiles
NPAIR = NT // 2    # 4 key-tile pairs (DoubleRow granularity)
DC = D // P        # 4 contraction chunks
QW = 512           # query chunk width (PSUM bank = 512 fp32)
QC = N // QW       # 2 query chunks
AUG = DH + 1       # v columns + constant column
XWDT = BF16
QKDT = BF16


class Job:
    __slots__ = ("min_g", "fn")

    def __init__(self, min_g, fn):
        self.min_g = min_g
        self.fn = fn


def pump(queue, g, budget):
    n = 0
    while queue and n < budget and queue[0].min_g <= g:
        queue.pop(0).fn()
        n += 1


def build_nc(reps=1, skip=()):
    skip = set(skip)
    nc = bacc.Bacc("TRN2", target_bir_lowering=False, debug=False,
                   num_devices=NCORES)
    d = {
        "xt": nc.dram_tensor("xt", [D, N], XWDT, kind="ExternalInput"),
        "wq": nc.dram_tensor("wq", [D, CQ], XWDT, kind="ExternalInput"),
        "wk": nc.dram_tensor("wk", [D, CQ], XWDT, kind="ExternalInput"),
        "wv": nc.dram_tensor("wv", [D, CV], XWDT, kind="ExternalInput"),
        "bq": nc.dram_tensor("bq", [P, HPC], F32, kind="ExternalInput"),
        "bk": nc.dram_tensor("bk", [P, HPC], F32, kind="ExternalInput"),
        "bvb": nc.dram_tensor("bvb", [P, CV], F32, kind="ExternalInput"),
        "o": nc.dram_tensor("o", [N, CV], F32, kind="ExternalOutput"),
    }
    with tile.TileContext(nc) as tc, ExitStack() as ctx:
        consts = ctx.enter_context(tc.tile_pool(name="consts", bufs=1))
        qk = ctx.enter_context(tc.tile_pool(name="qk", bufs=2))
        vaugp = ctx.enter_context(tc.tile_pool(name="vaugp", bufs=1))
        ep = ctx.enter_context(tc.tile_pool(name="ep", bufs=16))
        up = ctx.enter_context(tc.tile_pool(name="up", bufs=3))
        outp = ctx.enter_context(tc.tile_pool(name="outp", bufs=1))
        smallp = ctx.enter_context(tc.tile_pool(name="smallp", bufs=2))
        ps_proj = ctx.enter_context(
            tc.tile_pool(name="ps_proj", bufs=1, space="PSUM"))
        ps_score = ctx.enter_context(
            tc.tile_pool(name="ps_score", bufs=2, space="PSUM"))
        ps_pv = ctx.enter_context(
            tc.tile_pool(name="ps_pv", bufs=1, space="PSUM"))
        ps_tr = ctx.enter_context(
            tc.tile_pool(name="ps_tr", bufs=1, space="PSUM"))

        def body():
            # ---- input DMAs
            xt_sb, wq_sb, wk_sb, wv_sb = [], [], [], []
            for dc in range(DC):
                t = consts.tile([P, N], XWDT, tag=f"xt{dc}", name=f"xt{dc}")
                nc.sync.dma_start(t[:], d["xt"][dc * P:(dc + 1) * P, :])
                xt_sb.append(t)
                t = consts.tile([P, CQ], XWDT, tag=f"wq{dc}", name=f"wq{dc}")
                nc.sync.dma_start(t[:], d["wq"][dc * P:(dc + 1) * P, :])
                wq_sb.append(t)
                t = consts.tile([P, CQ], XWDT, tag=f"wk{dc}", name=f"wk{dc}")
                nc.sync.dma_start(t[:], d["wk"][dc * P:(dc + 1) * P, :])
                wk_sb.append(t)
            for dc in range(DC):
                t = consts.tile([P, CV], XWDT, tag=f"wv{dc}", name=f"wv{dc}")
                nc.sync.dma_start(t[:], d["wv"][dc * P:(dc + 1) * P, :])
                wv_sb.append(t)
            bq_sb = consts.tile([P, HPC], F32, tag="bq", name="bq")
            nc.sync.dma_start(bq_sb[:], d["bq"][:])
            bk_sb = consts.tile([P, HPC], F32, tag="bk", name="bk")
            nc.sync.dma_start(bk_sb[:], d["bk"][:])
            bvb_sb = consts.tile([P, CV], F32, tag="bvb", name="bvb")
            nc.sync.dma_start(bvb_sb[:], d["bvb"][:])
            ident = consts.tile([P, P], F32, tag="ident", name="ident")
            make_identity(nc, ident[:])

            # ---- v projection + augmentation (+1 / -2 constant columns)
            # vaug[nt][half]: [128, HPC*AUG] bf16, head h at [h*AUG : +AUG]
            vaug = [[vaugp.tile([P, HPC * AUG], BF16,
                                tag=f"vaug{nt}_{half}",
                                name=f"vaug{nt}_{half}")
                     for half in range(2)] for nt in range(NT)]
            vproj_q = []

            def mk_vproj(nt):
                def job():
                    ps = ps_proj.tile([P, CV], F32, tag="proj",
                                      name=f"ps_v{nt}")
                    for dc in range(DC):
                        nc.tensor.matmul(
                            ps[:],
                            xt_sb[dc][:, nt * P:(nt + 1) * P],
                            wv_sb[dc][:],
                            start=(dc == 0), stop=(dc == DC - 1))
                    psv = ps[:].rearrange("p (h a) -> p h a", a=DH)
                    bvv = bvb_sb[:].rearrange("p (h a) -> p h a", a=DH)
                    for half in range(2):
                        tv = vaug[nt][half][:].rearrange(
                            "p (h a) -> p h a", a=AUG)
                        nc.vector.tensor_add(tv[:, :, 0:DH], psv, bvv)
                        nc.vector.memset(tv[:, :, DH:AUG],
                                         1.0 if half == 0 else -2.0)
                return job
            for nt in range(NT):
                vproj_q.append(Job(-1, mk_vproj(nt)))

            # ---- output staging tiles
            ostage = [outp.tile([P, CV], F32, tag=f"ost{q}", name=f"ost{q}")
                      for q in range(NT)]

            proj_q, pv_q, fin_q = [], [], []

            # ---- projection jobs for one head's q/k (16 PE MMs; the 4th
            # of each accumulation group also does the bias add)
            def push_proj_jobs(h, qt, kt_, min_g):
                for qc in range(QC):
                    for w_sb, b_sb, dest, pfx in ((wq_sb, bq_sb, qt, "q"),
                                                  (wk_sb, bk_sb, kt_, "k")):
                        ps = [None]

                        def mk(dc, w_sb=w_sb, b_sb=b_sb, dest=dest, pfx=pfx,
                               qc=qc, ps=ps):
                            def job():
                                if dc == 0:
                                    ps[0] = ps_proj.tile(
                                        [P, QW], F32, tag="proj",
                                        name=f"ps_{pfx}{h}_{qc}")
                                nc.tensor.matmul(
                                    ps[0][:],
                                    w_sb[dc][:, h * P:(h + 1) * P],
                                    xt_sb[dc][:, qc * QW:(qc + 1) * QW],
                                    start=(dc == 0), stop=(dc == DC - 1))
                                if dc == DC - 1:
                                    nc.vector.tensor_scalar_add(
                                        dest[:, qc * QW:(qc + 1) * QW],
                                        ps[0][:], b_sb[:, h:h + 1])
                            return job
                        for dc in range(DC):
                            if "proj" not in skip:
                                proj_q.append(Job(min_g, mk(dc)))

            # ---- finish jobs for head h (one per query tile)
            def push_fin_jobs(h, u_pair, min_g):
                u1, u2 = u_pair

                def mk(qt_i):
                    def job():
                        tr = ps_tr.tile([P, 2 * AUG], F32, tag="tr",
                                        name=f"ps_tr{h}_{qt_i}")
                        nc.tensor.transpose(
                            tr[:, 0:AUG],
                            u1[0:AUG, qt_i * P:(qt_i + 1) * P],
                            ident[0:AUG, 0:AUG])
                        nc.tensor.transpose(
                            tr[:, AUG:2 * AUG],
                            u2[0:AUG, qt_i * P:(qt_i + 1) * P],
                            ident[0:AUG, 0:AUG])
                        rr = smallp.tile([P, 2], F32, tag="rr",
                                         name=f"rr_{h}_{qt_i}")
                        trv = tr[:].rearrange("p (c a) -> p c a", a=AUG)
                        nc.vector.reciprocal(rr[:], trv[:, :, DH])
                        o1 = smallp.tile([P, DH], F32, tag="o1",
                                         name=f"o1_{h}{qt_i}")
                        o2 = smallp.tile([P, DH], F32, tag="o2",
                                         name=f"o2_{h}{qt_i}")
                        nc.vector.tensor_scalar_mul(
                            o1[:], tr[:, 0:DH], rr[:, 0:1])
                        nc.vector.tensor_scalar_mul(
                            o2[:], tr[:, AUG:AUG + DH], rr[:, 1:2])
                        nc.vector.tensor_add(
                            ostage[qt_i][:, h * DH:(h + 1) * DH], o1[:], o2[:])
                        nc.sync.dma_start(
                            d["o"][qt_i * P:(qt_i + 1) * P,
                                   h * DH:(h + 1) * DH],
                            ostage[qt_i][:, h * DH:(h + 1) * DH])
                    return job
                for qt_i in range(NT):
                    if "fin" not in skip:
                        fin_q.append(Job(min_g, mk(qt_i)))

            # ---- PV jobs for head h: 32 MMs, groups (half,qc) accumulate
            # kt 0..7; half-sequential so only two 1-bank accumulators are
            # open (half 1 drains into the next head's periods).
            # e tile [128, 2048]: [h0q0|h1q0|h0q1|h1q1] x 512.
            def push_pv_jobs(h, etiles, u_tiles, base_g):
                pvst = {}

                def mk(half, kt, qc):
                    def job():
                        if kt == 0:
                            pvst[(half, qc)] = ps_pv.tile(
                                [AUG, QW], F32, tag=f"pv{qc}",
                                name=f"ps_pv{h}_{half}_{qc}")
                        pvt = pvst[(half, qc)]
                        nc.tensor.matmul(
                            pvt[:],
                            vaug[kt][half][:, h * AUG:(h + 1) * AUG],
                            etiles[kt][:, (half + 2 * qc) * QW:
                                       (half + 2 * qc) * QW + QW],
                            start=(kt == 0), stop=(kt == NT - 1))
                        if kt == NT - 1:
                            nc.vector.tensor_copy(
                                u_tiles[half][:, qc * QW:(qc + 1) * QW],
                                pvt[:])
                            if half == 1 and qc == QC - 1:
                                push_fin_jobs(h, (u_tiles[0], u_tiles[1]),
                                              cur_g[0] + 2)
                    return job
                for half in range(2):
                    for kt in range(NT):
                        for qc in range(QC):
                            # e(kt) ready after the ACT emitted at
                            # period base_g + kt
                            if "pv" not in skip:
                                pv_q.append(Job(base_g + kt + 2,
                                                mk(half, kt, qc)))

            cur_g = [0]

            # ---- head 0 projections up front
            qt, kt_, = (qk.tile([P, N], QKDT, tag="qt", name="qt0"),
                        qk.tile([P, N], QKDT, tag="kt", name="kt0"))
            if "proj" in skip:
                nc.vector.memset(qt[:], 0.0)
                nc.vector.memset(kt_[:], 0.0)
            push_proj_jobs(0, qt, kt_, min_g=-1)
            while proj_q and proj_q[0].min_g <= -1:
                proj_q.pop(0).fn()

            heads = {0: (qt, kt_)}
            for g in range(HPC * NT):
                h, kt = g // NT, g % NT
                cur_g[0] = g
                if kt == 0:
                    # allocate next head's qt/kt and queue its projections
                    if h + 1 < HPC:
                        nqt = qk.tile([P, N], QKDT, tag="qt", name=f"qt{h+1}")
                        nkt = qk.tile([P, N], QKDT, tag="kt", name=f"kt{h+1}")
                        heads[h + 1] = (nqt, nkt)
                        if "proj" in skip:
                            nc.vector.memset(nqt[:], 0.0)
                            nc.vector.memset(nkt[:], 0.0)
                        push_proj_jobs(h + 1, nqt, nkt, min_g=g)
                    # e tiles + u tiles + pv jobs for this head
                    etiles = [ep.tile([P, 2048], BF16, tag="e",
                                      name=f"e{h}_{k}")
                              for k in range(NT)]
                    if "score" in skip and "pv" not in skip:
                        for t in etiles:
                            nc.vector.memset(t[:], 1.0)
                    u_tiles = [up.tile([AUG, N], F32, tag=f"u{hf}",
                                       name=f"u{h}_{hf}")
                               for hf in range(2)]
                    push_pv_jobs(h, etiles, u_tiles, base_g=g)
                qt, kt_ = heads[h]
                for qc in range(QC):
                    if "score" in skip:
                        break
                    ps = ps_score.tile([P, 1024], F32, tag="score",
                                       name=f"s{h}_{kt}_{qc}")
                    for half in range(2):
                        for _rep in range(2 if "dup" in skip else 1):
                            nc.tensor.matmul(
                                ps[:, half * QW:half * QW + QW],
                                kt_[half * DH:(half + 1) * DH,
                                    kt * P:(kt + 1) * P],
                                qt[half * DH:(half + 1) * DH,
                                   qc * QW:(qc + 1) * QW],
                                start=True, stop=True)
                    if "act" not in skip:
                        nc.scalar.activation(
                            etiles[kt][:, qc * 1024:(qc + 1) * 1024], ps[:],
                            mybir.ActivationFunctionType.Exp, scale=SCALE)
                pump(vproj_q, g, 1)
                pump(pv_q, g, 6)
                pump(proj_q, g, 2)
                if kt >= 1:
                    pump(fin_q, g, 1)
            cur_g[0] = 10 ** 6
            while pv_q:
                pv_q.pop(0).fn()
            while fin_q:
                fin_q.pop(0).fn()

        if reps == 1:
            body()
        else:
            with tc.For_i(0, reps, 1,
                          hint_engines=(mybir.EngineType.PE,
                                        mybir.EngineType.DVE)):
                body()

    nc.compile()
    return nc


_NC_CACHE = {}


def get_nc(reps=1):
    if reps not in _NC_CACHE:
        _NC_CACHE[reps] = build_nc(reps)
    return _NC_CACHE[reps]


def shard_inputs(inputs):
    import ml_dtypes
    xw_np = (np.dtype(ml_dtypes.bfloat16)
             if XWDT == mybir.dt.bfloat16 else np.float32)
    x = np.asarray(inputs["x"], dtype=np.float32)
    Wq = np.asarray(inputs["Wq"], dtype=np.float32)
    bq = np.asarray(inputs["bq"], dtype=np.float32)
    Wk = np.asarray(inputs["Wk"], dtype=np.float32)
    bk = np.asarray(inputs["bk"], dtype=np.float32)
    Wv = np.asarray(inputs["Wv"], dtype=np.float32)
    bv = np.asarray(inputs["bv"], dtype=np.float32)
    in_maps = []
    for c in range(NCORES):
        b = c // 2
        h0 = (c % 2) * HPC
        cq0 = h0 * 2 * DH          # 0 or 512 in the q/k projection cols
        cv0 = h0 * DH              # 0 or 256 in the v cols
        in_maps.append({
            "xt": np.ascontiguousarray(x[b].T).astype(xw_np),
            "wq": np.ascontiguousarray(Wq[:, cq0:cq0 + CQ]).astype(xw_np),
            "wk": np.ascontiguousarray(Wk[:, cq0:cq0 + CQ]).astype(xw_np),
            "wv": np.ascontiguousarray(Wv[:, cv0:cv0 + CV]).astype(xw_np),
            "bq": np.ascontiguousarray(bq[cq0:cq0 + CQ].reshape(HPC, P).T),
            "bk": np.ascontiguousarray(bk[cq0:cq0 + CQ].reshape(HPC, P).T),
            "bvb": np.ascontiguousarray(
                np.broadcast_to(bv[cv0:cv0 + CV], (P, CV))),
        })
    return in_maps


def assemble_output(results):
    out = np.empty((B, N, D), dtype=np.float32)
    for c in range(NCORES):
        b = c // 2
        g = c % 2
        out[b, :, g * CV:(g + 1) * CV] = results[c]["o"]
    return out


def kernel(**inputs):
    nc = get_nc(1)
    in_maps = shard_inputs(inputs)
    res = bass_utils.run_bass_kernel_spmd(
        nc, in_maps, core_ids=list(range(NCORES)))
    return assemble_output(res.results)


# revision 24
# speedup vs baseline: 1.6573x; 1.1250x over previous
"""Differential multi-head attention Trainium2 Bass kernel.

Problem: B=4, N=1024, D=512, H=8 heads, DH=64. LAM=0.5.
  q = (x@Wq+bq)  -> [B,H,N,2*DH], halves q1,q2 (same for k)
  a_i = softmax(q_i@k_i^T / sqrt(DH)); attn = a1 - LAM*a2; out = attn@v

Sharding: 8 cores; core c handles batch b=c//2 and heads h0..h0+3 with
h0=(c%2)*4 (batch + head-group parallel). Weights column-sharded by head.

v3 schedule (all bf16 — fp8 was tried and rejected: operand rounding
error transfers ~1:1 to output relative error, and fp8e4m3's ~4-6%
blows the 2e-2 gate; bf16's ~0.4% gives ~3e-3).

Measured engine model on this platform: a 512-free matmul costs
~0.4-0.6us (PE effectively ~1.2GHz sustained + ~150-200ns/instr);
total PE streaming is the bottleneck, ACT exp (~60us busy at
2048-wide) hides underneath.
  - score matmuls for the two softmax halves are emitted adjacently;
    their lhsT base partitions (0/64) land on different PE row groups
    and run concurrently (~1.4x measured).
  - exp is one 2048-wide ACTIVATE per key tile covering both halves,
    written as bf16 into per-kt e tiles [128,2048].
  - PV: per-kt matmuls; v is augmented with a +1/-2 constant column
    producing softmax denominators in the same accumulation
    (u1/s1 + u2/(-2 s2) = a1@v - 0.5 a2@v).
  - one global period loop (32 periods = 4 heads x 8 key tiles); each
    period emits 4 score MMs + 1 exp, then pumps FIFO job queues with
    per-period budgets: PV 4, next-head projection 2, finish 1.
    PV groups (half,qc) accumulate 8 kt-MMs half-sequentially (the
    second half drains into the next head's periods) so only two
    1-bank PSUM accumulators are ever open.
PSUM (8 banks): scores 1x[128,2048]=4, pv 2x[65,512]=2, proj 1, tr 1.
"""
import hashlib
import os
import sys

sys.path.insert(0, "/opt/trn_rl_repo")

# The libneuronxla NEFF cache keys on the HLO module hash, which does NOT
# change when only the Bass kernel body changes (the custom call carries a
# content-free token) — a shared cache silently reuses stale NEFFs from
# earlier kernel versions. Key the cache dir by this file's content hash.
with open(__file__, "rb") as _f:
    _KSRC = _f.read()
os.environ["NEURON_COMPILE_CACHE_URL"] = (
    f"/tmp/neuron-cache-{hashlib.sha1(_KSRC).hexdigest()[:12]}"
)

from contextlib import ExitStack

import numpy as np

import concourse.bass as bass
import concourse.mybir as mybir
import concourse.tile as tile
from concourse import bacc, bass_utils
from concourse.masks import make_identity

F32 = mybir.dt.float32
BF16 = mybir.dt.bfloat16

B, N, D, H = 4, 1024, 512, 8
DH = 64            # per-head dim for v and per q/k half
HPC = 4            # heads per core
LAM = 0.5
SCALE = 0.125      # 1/sqrt(DH)
NCORES = 8
CQ = HPC * 2 * DH  # 512 projection cols per core for q/k
CV = HPC * DH      # 256 projection cols per core for v
P = 128
NT = N // P        # 8 key tiles
NPAIR = NT // 2    # 4 key-tile pairs (DoubleRow granularity)
DC = D // P        # 4 contraction chunks
QW = 512           # query chunk width (PSUM bank = 512 fp32)
QC = N // QW       # 2 query chunks
AUG = DH + 1       # v columns + constant column
XWDT = BF16
QKDT = BF16


class Job:
    __slots__ = ("min_g", "fn")

    def __init__(self, min_g, fn):
        self.min_g = min_g
        self.fn = fn


def pump(queue, g, budget):
    n = 0
    while queue and n < budget and queue[0].min_g <= g:
        queue.pop(0).fn()
        n += 1


def build_nc(reps=1, skip=()):
    skip = set(skip)
    nc = bacc.Bacc("TRN2", target_bir_lowering=False, debug=False,
                   num_devices=NCORES)
    d = {
        "xt": nc.dram_tensor("xt", [D, N], XWDT, kind="ExternalInput"),
        "wq": nc.dram_tensor("wq", [D, CQ], XWDT, kind="ExternalInput"),
        "wk": nc.dram_tensor("wk", [D, CQ], XWDT, kind="ExternalInput"),
        "wv": nc.dram_tensor("wv", [D, CV], XWDT, kind="ExternalInput"),
        "bq": nc.dram_tensor("bq", [P, HPC], F32, kind="ExternalInput"),
        "bk": nc.dram_tensor("bk", [P, HPC], F32, kind="ExternalInput"),
        "bvb": nc.dram_tensor("bvb", [P, CV], F32, kind="ExternalInput"),
        "o": nc.dram_tensor("o", [N, CV], F32, kind="ExternalOutput"),
    }
    with tile.TileContext(nc) as tc, ExitStack() as ctx:
        consts = ctx.enter_context(tc.tile_pool(name="consts", bufs=1))
        qk = ctx.enter_context(tc.tile_pool(name="qk", bufs=2))
        vaugp = ctx.enter_context(tc.tile_pool(name="vaugp", bufs=1))
        ep = ctx.enter_context(tc.tile_pool(name="ep", bufs=16))
        up = ctx.enter_context(tc.tile_pool(name="up", bufs=3))
        outp = ctx.enter_context(tc.tile_pool(name="outp", bufs=1))
        smallp = ctx.enter_context(tc.tile_pool(name="smallp", bufs=2))
        ps_proj = ctx.enter_context(
            tc.tile_pool(name="ps_proj", bufs=1, space="PSUM"))
        ps_score = ctx.enter_context(
            tc.tile_pool(name="ps_score", bufs=2, space="PSUM"))
        ps_pv = ctx.enter_context(
            tc.tile_pool(name="ps_pv", bufs=1, space="PSUM"))
        ps_tr = ctx.enter_context(
            tc.tile_pool(name="ps_tr", bufs=1, space="PSUM"))

        def body():
            # ---- input DMAs
            xt_sb, wq_sb, wk_sb, wv_sb = [], [], [], []
            for dc in range(DC):
                t = consts.tile([P, N], XWDT, tag=f"xt{dc}", name=f"xt{dc}")
                nc.sync.dma_start(t[:], d["xt"][dc * P:(dc + 1) * P, :])
                xt_sb.append(t)
                t = consts.tile([P, CQ], XWDT, tag=f"wq{dc}", name=f"wq{dc}")
                nc.sync.dma_start(t[:], d["wq"][dc * P:(dc + 1) * P, :])
                wq_sb.append(t)
                t = consts.tile([P, CQ], XWDT, tag=f"wk{dc}", name=f"wk{dc}")
                nc.sync.dma_start(t[:], d["wk"][dc * P:(dc + 1) * P, :])
                wk_sb.append(t)
            for dc in range(DC):
                t = consts.tile([P, CV], XWDT, tag=f"wv{dc}", name=f"wv{dc}")
                nc.sync.dma_start(t[:], d["wv"][dc * P:(dc + 1) * P, :])
                wv_sb.append(t)
            bq_sb = consts.tile([P, HPC], F32, tag="bq", name="bq")
            nc.sync.dma_start(bq_sb[:], d["bq"][:])
            bk_sb = consts.tile([P, HPC], F32, tag="bk", name="bk")
            nc.sync.dma_start(bk_sb[:], d["bk"][:])
            bvb_sb = consts.tile([P, CV], F32, tag="bvb", name="bvb")
            nc.sync.dma_start(bvb_sb[:], d["bvb"][:])
            ident = consts.tile([P, P], F32, tag="ident", name="ident")
            make_identity(nc, ident[:])

            # ---- v projection + augmentation (+1 / -2 constant columns)
            # vaug[nt][half]: [128, HPC*AUG] bf16, head h at [h*AUG : +AUG]
            vaug = [[vaugp.tile([P, HPC * AUG], BF16,
                                tag=f"vaug{nt}_{half}",
                                name=f"vaug{nt}_{half}")
                     for half in range(2)] for nt in range(NT)]
            vproj_q = []

            def mk_vproj(nt):
                def job():
                    ps = ps_proj.tile([P, CV], F32, tag="proj",
                                      name=f"ps_v{nt}")
                    for dc in range(DC):
                        nc.tensor.matmul(
                            ps[:],
                            xt_sb[dc][:, nt * P:(nt + 1) * P],
                            wv_sb[dc][:],
                            start=(dc == 0), stop=(dc == DC - 1))
                    psv = ps[:].rearrange("p (h a) -> p h a", a=DH)
                    bvv = bvb_sb[:].rearrange("p (h a) -> p h a", a=DH)
                    for half in range(2):
                        tv = vaug[nt][half][:].rearrange(
                            "p (h a) -> p h a", a=AUG)
                        nc.vector.tensor_add(tv[:, :, 0:DH], psv, bvv)
                        nc.vector.memset(tv[:, :, DH:AUG],
                                         1.0 if half == 0 else -2.0)
                return job
            for nt in range(NT):
                vproj_q.append(Job(-1, mk_vproj(nt)))

            # ---- output staging tiles
            ostage = [outp.tile([P, CV], F32, tag=f"ost{q}", name=f"ost{q}")
                      for q in range(NT)]

            proj_q, pv_q, fin_q = [], [], []

            # ---- projection jobs for one head's q/k (16 PE MMs; the 4th
            # of each accumulation group also does the bias add)
            def push_proj_jobs(h, qt, kt_, min_g):
                for qc in range(QC):
                    for w_sb, b_sb, dest, pfx in ((wq_sb, bq_sb, qt, "q"),
                                                  (wk_sb, bk_sb, kt_, "k")):
                        ps = [None]

                        def mk(dc, w_sb=w_sb, b_sb=b_sb, dest=dest, pfx=pfx,
                               qc=qc, ps=ps):
                            def job():
                                if dc == 0:
                                    ps[0] = ps_proj.tile(
                                        [P, QW], F32, tag="proj",
                                        name=f"ps_{pfx}{h}_{qc}")
                                nc.tensor.matmul(
                                    ps[0][:],
                                    w_sb[dc][:, h * P:(h + 1) * P],
                                    xt_sb[dc][:, qc * QW:(qc + 1) * QW],
                                    start=(dc == 0), stop=(dc == DC - 1))
                                if dc == DC - 1:
                                    nc.vector.tensor_scalar_add(
                                        dest[:, qc * QW:(qc + 1) * QW],
                                        ps[0][:], b_sb[:, h:h + 1])
                            return job
                        for dc in range(DC):
                            if "proj" not in skip:
                                proj_q.append(Job(min_g, mk(dc)))

            # ---- finish jobs for head h (a PAIR of query tiles per job
            # sharing one tr bank allocation, halving the tr WAR chain)
            def push_fin_jobs(h, u_pair, min_g):
                u1, u2 = u_pair

                def mk(q0):
                    def job():
                        tr = ps_tr.tile([P, 4 * AUG], F32, tag="tr",
                                        name=f"ps_tr{h}_{q0}")
                        for j, qt_i in enumerate((q0, q0 + 1)):
                            nc.tensor.transpose(
                                tr[:, (2 * j) * AUG:(2 * j + 1) * AUG],
                                u1[0:AUG, qt_i * P:(qt_i + 1) * P],
                                ident[0:AUG, 0:AUG])
                            nc.tensor.transpose(
                                tr[:, (2 * j + 1) * AUG:(2 * j + 2) * AUG],
                                u2[0:AUG, qt_i * P:(qt_i + 1) * P],
                                ident[0:AUG, 0:AUG])
                        rr = smallp.tile([P, 4], F32, tag="rr",
                                         name=f"rr_{h}_{q0}")
                        trv = tr[:].rearrange("p (c a) -> p c a", a=AUG)
                        nc.vector.reciprocal(rr[:], trv[:, :, DH])
                        for j, qt_i in enumerate((q0, q0 + 1)):
                            o1 = smallp.tile([P, DH], F32, tag="o1",
                                             name=f"o1_{h}{qt_i}")
                            o2 = smallp.tile([P, DH], F32, tag="o2",
                                             name=f"o2_{h}{qt_i}")
                            nc.vector.tensor_scalar_mul(
                                o1[:], tr[:, (2 * j) * AUG:
                                           (2 * j) * AUG + DH],
                                rr[:, 2 * j:2 * j + 1])
                            nc.vector.tensor_scalar_mul(
                                o2[:], tr[:, (2 * j + 1) * AUG:
                                           (2 * j + 1) * AUG + DH],
                                rr[:, 2 * j + 1:2 * j + 2])
                            nc.vector.tensor_add(
                                ostage[qt_i][:, h * DH:(h + 1) * DH],
                                o1[:], o2[:])
                            nc.sync.dma_start(
                                d["o"][qt_i * P:(qt_i + 1) * P,
                                       h * DH:(h + 1) * DH],
                                ostage[qt_i][:, h * DH:(h + 1) * DH])
                    return job
                for q0 in range(0, NT, 2):
                    if "fin" not in skip:
                        fin_q.append(Job(min_g, mk(q0)))

            # ---- PV jobs for head h: 32 MMs, groups (half,qc) accumulate
            # kt 0..7; half-sequential so only two 1-bank accumulators are
            # open (half 1 drains into the next head's periods).
            # e tile [128, 2048]: [h0q0|h1q0|h0q1|h1q1] x 512.
            def push_pv_jobs(h, etiles, u_tiles, base_g):
                pvst = {}

                def mk(half, kt, qc):
                    def job():
                        if kt == 0:
                            pvst[(half, qc)] = ps_pv.tile(
                                [AUG, QW], F32, tag=f"pv{qc}",
                                name=f"ps_pv{h}_{half}_{qc}")
                        pvt = pvst[(half, qc)]
                        nc.tensor.matmul(
                            pvt[:],
                            vaug[kt][half][:, h * AUG:(h + 1) * AUG],
                            etiles[kt][:, (half + 2 * qc) * QW:
                                       (half + 2 * qc) * QW + QW],
                            start=(kt == 0), stop=(kt == NT - 1))
                        if kt == NT - 1:
                            nc.vector.tensor_copy(
                                u_tiles[half][:, qc * QW:(qc + 1) * QW],
                                pvt[:])
                            if half == 1 and qc == QC - 1:
                                push_fin_jobs(h, (u_tiles[0], u_tiles[1]),
                                              cur_g[0] + 2)
                    return job
                for half in range(2):
                    for kt in range(NT):
                        for qc in range(QC):
                            # e(kt) ready after the ACT emitted at
                            # period base_g + kt
                            if "pv" not in skip:
                                pv_q.append(Job(base_g + kt + 3,
                                                mk(half, kt, qc)))

            cur_g = [0]
            pend_act = []

            # ---- head 0 projections up front
            qt, kt_, = (qk.tile([P, N], QKDT, tag="qt", name="qt0"),
                        qk.tile([P, N], QKDT, tag="kt", name="kt0"))
            if "proj" in skip:
                nc.vector.memset(qt[:], 0.0)
                nc.vector.memset(kt_[:], 0.0)
            push_proj_jobs(0, qt, kt_, min_g=-1)
            while proj_q and proj_q[0].min_g <= -1:
                proj_q.pop(0).fn()

            heads = {0: (qt, kt_)}
            for g in range(HPC * NT):
                h, kt = g // NT, g % NT
                cur_g[0] = g
                if kt == 0:
                    # allocate next head's qt/kt and queue its projections
                    if h + 1 < HPC:
                        nqt = qk.tile([P, N], QKDT, tag="qt", name=f"qt{h+1}")
                        nkt = qk.tile([P, N], QKDT, tag="kt", name=f"kt{h+1}")
                        heads[h + 1] = (nqt, nkt)
                        if "proj" in skip:
                            nc.vector.memset(nqt[:], 0.0)
                            nc.vector.memset(nkt[:], 0.0)
                        push_proj_jobs(h + 1, nqt, nkt, min_g=g)
                    # e tiles + u tiles + pv jobs for this head
                    etiles = [ep.tile([P, 2048], BF16, tag="e",
                                      name=f"e{h}_{k}")
                              for k in range(NT)]
                    if "score" in skip and "pv" not in skip:
                        for t in etiles:
                            nc.vector.memset(t[:], 1.0)
                    u_tiles = [up.tile([AUG, N], F32, tag=f"u{hf}",
                                       name=f"u{h}_{hf}")
                               for hf in range(2)]
                    push_pv_jobs(h, etiles, u_tiles, base_g=g)
                qt, kt_ = heads[h]
                for a in pend_act:
                    a()
                pend_act.clear()
                for qc in range(QC):
                    if "score" in skip:
                        break
                    ps = ps_score.tile([P, 1024], F32, tag="score",
                                       name=f"s{h}_{kt}_{qc}")
                    for half in range(2):
                        for _rep in range(2 if "dup" in skip else 1):
                            nc.tensor.matmul(
                                ps[:, half * QW:half * QW + QW],
                                kt_[half * DH:(half + 1) * DH,
                                    kt * P:(kt + 1) * P],
                                qt[half * DH:(half + 1) * DH,
                                   qc * QW:(qc + 1) * QW],
                                start=True, stop=True)
                    if "act" not in skip:
                        def mk_act(dst=etiles[kt], c=qc, src=ps):
                            def a():
                                nc.scalar.activation(
                                    dst[:, c * 1024:(c + 1) * 1024], src[:],
                                    mybir.ActivationFunctionType.Exp,
                                    scale=SCALE)
                            return a
                        pend_act.append(mk_act())
                pump(vproj_q, g, 1)
                pump(pv_q, g, 6)
                pump(proj_q, g, 2)
                if kt % 2 == 1:
                    pump(fin_q, g, 1)
            cur_g[0] = 10 ** 6
            for a in pend_act:
                a()
            pend_act.clear()
            while pv_q:
                pv_q.pop(0).fn()
            while fin_q:
                fin_q.pop(0).fn()

        if reps == 1:
            body()
        else:
            with tc.For_i(0, reps, 1,
                          hint_engines=(mybir.EngineType.PE,
                                        mybir.EngineType.DVE)):
                body()

    nc.compile()
    return nc


_NC_CACHE = {}


def get_nc(reps=1):
    if reps not in _NC_CACHE:
        _NC_CACHE[reps] = build_nc(reps)
    return _NC_CACHE[reps]


def shard_inputs(inputs):
    import ml_dtypes
    xw_np = (np.dtype(ml_dtypes.bfloat16)
             if XWDT == mybir.dt.bfloat16 else np.float32)
    x = np.asarray(inputs["x"], dtype=np.float32)
    Wq = np.asarray(inputs["Wq"], dtype=np.float32)
    bq = np.asarray(inputs["bq"], dtype=np.float32)
    Wk = np.asarray(inputs["Wk"], dtype=np.float32)
    bk = np.asarray(inputs["bk"], dtype=np.float32)
    Wv = np.asarray(inputs["Wv"], dtype=np.float32)
    bv = np.asarray(inputs["bv"], dtype=np.float32)
    in_maps = []
    for c in range(NCORES):
        b = c // 2
        h0 = (c % 2) * HPC
        cq0 = h0 * 2 * DH          # 0 or 512 in the q/k projection cols
        cv0 = h0 * DH              # 0 or 256 in the v cols
        in_maps.append({
            "xt": np.ascontiguousarray(x[b].T).astype(xw_np),
            "wq": np.ascontiguousarray(Wq[:, cq0:cq0 + CQ]).astype(xw_np),
            "wk": np.ascontiguousarray(Wk[:, cq0:cq0 + CQ]).astype(xw_np),
            "wv": np.ascontiguousarray(Wv[:, cv0:cv0 + CV]).astype(xw_np),
            "bq": np.ascontiguousarray(bq[cq0:cq0 + CQ].reshape(HPC, P).T),
            "bk": np.ascontiguousarray(bk[cq0:cq0 + CQ].reshape(HPC, P).T),
            "bvb": np.ascontiguousarray(
                np.broadcast_to(bv[cv0:cv0 + CV], (P, CV))),
        })
    return in_maps


def assemble_output(results):
    out = np.empty((B, N, D), dtype=np.float32)
    for c in range(NCORES):
        b = c // 2
        g = c % 2
        out[b, :, g * CV:(g + 1) * CV] = results[c]["o"]
    return out


def kernel(**inputs):
    nc = get_nc(1)
    in_maps = shard_inputs(inputs)
    res = bass_utils.run_bass_kernel_spmd(
        nc, in_maps, core_ids=list(range(NCORES)))
    return assemble_output(res.results)
